# revision 1
# baseline (speedup 1.0000x reference)
"""ChildSumTreeLSTM on a complete binary tree (N=8191), 8-core Trainium2.

Strategy: the heap-ordered complete tree = 7 top nodes + 8 independent
1023-node subtrees. Each NeuronCore gets one subtree (tree-level
parallelism), computes the batched x-projections (emb lookup done on host,
projections as dense matmuls on the PE) and a level-synchronous scan
(leaves -> subtree root) with everything feature-major [256 feats x nodes].
One 16KB AllGather shares the 8 subtree roots; the top 3 levels are
computed redundantly on every core. Output read from core 0.
"""

import numpy as np

import concourse.bass as bass
import concourse.tile as tile
from concourse import mybir
from concourse.bass_utils import run_bass_kernel_spmd

F32 = mybir.dt.float32
BF16 = mybir.dt.bfloat16
AFT = mybir.ActivationFunctionType

N_NODES = 8191
D = 256
M = 256
NCOL = 1024  # col 0 pad + 1023 subtree cols (leaves at 512..1023)
SUB_LEVELS = 10  # subtree: 512 leaves ... 1 root
USE_F32R = True  # flip to use TF32-like fast fp32 matmuls


def _split_excess_waits(nc, max_waits=1):
    """walrus in this container allows only 1 sync-wait per instruction.

    Tile can attach several; hoist the extras onto injected same-engine NOPs
    immediately preceding the instruction (same blocking semantics)."""
    k = 0
    for f in nc.m.functions:
        for bb in f.blocks:
            out = []
            changed = False
            for ins in bb.instructions:
                si = ins.sync_info
                w = list(si.on_wait) if si and si.on_wait else []
                if len(w) > max_waits:
                    hoist, keep = w[:-max_waits], w[-max_waits:]
                    for sw in hoist:
                        nop = mybir.InstNoOp(name=f"whoist{k}", ins=[], outs=[])
                        k += 1
                        nop.engine = ins.engine
                        nop.sync_info = mybir.SyncInfo(on_wait=[sw], on_update=[])
                        out.append(nop)
                    si.on_wait = keep
                    changed = True
                out.append(ins)
            if changed:
                bb.instructions = out


def _mmcast(ap):
    return ap.bitcast(mybir.dt.float32r) if USE_F32R else ap


def _build_module():
    nc = bass.Bass(num_devices=8)

    xT = nc.dram_tensor("xT", [D, NCOL], BF16, kind="ExternalInput")
    wcT = nc.dram_tensor("wcT", [D, 1024], BF16, kind="ExternalInput")
    wiouhT = nc.dram_tensor("wiouhT", [M, 768], BF16, kind="ExternalInput")
    wfhT = nc.dram_tensor("wfhT", [M, 256], BF16, kind="ExternalInput")
    b_iou_int = nc.dram_tensor("b_iou_int", [128, 6], F32, kind="ExternalInput")
    b_iou_leaf = nc.dram_tensor("b_iou_leaf", [128, 6], F32, kind="ExternalInput")
    b_f_int = nc.dram_tensor("b_f_int", [128, 2], F32, kind="ExternalInput")
    b_f_leaf = nc.dram_tensor("b_f_leaf", [128, 2], F32, kind="ExternalInput")
    out = nc.dram_tensor("out", [512, 8], F32, kind="ExternalOutput")


    with tile.TileContext(nc) as tc:
        with (
            tc.tile_pool(name="consts", bufs=1) as consts,
            tc.tile_pool(name="tmps", bufs=3) as tmps,
            tc.tile_pool(name="scan_psum", bufs=1, space="PSUM") as spsum,
        ):
            # ---- resident SBUF tensors ----
            sb_xT = []
            for kt in range(2):
                t = consts.tile([128, NCOL], BF16, tag=f"xT{kt}")
                nc.sync.dma_start(out=t[:], in_=xT[128 * kt : 128 * (kt + 1), :])
                sb_xT.append(t)
            sb_wcT = []
            for kt in range(2):
                t = consts.tile([128, 1024], BF16, tag=f"wcT{kt}")
                nc.sync.dma_start(out=t[:], in_=wcT[128 * kt : 128 * (kt + 1), :])
                sb_wcT.append(t)
            sb_wiouhT = []
            for kt in range(2):
                t = consts.tile([128, 768], BF16, tag=f"wiouhT{kt}")
                nc.sync.dma_start(out=t[:], in_=wiouhT[128 * kt : 128 * (kt + 1), :])
                sb_wiouhT.append(t)
            sb_wfhT = []
            for kt in range(2):
                t = consts.tile([128, 256], BF16, tag=f"wfhT{kt}")
                nc.sync.dma_start(out=t[:], in_=wfhT[128 * kt : 128 * (kt + 1), :])
                sb_wfhT.append(t)
            sb_biou_i = consts.tile([128, 6], F32, tag="biou_i")
            nc.sync.dma_start(out=sb_biou_i[:], in_=b_iou_int[:])
            sb_biou_l = consts.tile([128, 6], F32, tag="biou_l")
            nc.sync.dma_start(out=sb_biou_l[:], in_=b_iou_leaf[:])
            sb_bf_i = consts.tile([128, 2], F32, tag="bf_i")
            nc.sync.dma_start(out=sb_bf_i[:], in_=b_f_int[:])
            sb_bf_l = consts.tile([128, 2], F32, tag="bf_l")
            nc.sync.dma_start(out=sb_bf_l[:], in_=b_f_leaf[:])

            # IOUXFX[F][p, c]: feature 128F+p for node col c.
            # F 0..1 = i, 2..3 = o, 4..5 = u, 6..7 = fx  (no biases folded)
            IOUXFX = [consts.tile([128, NCOL], F32, tag=f"iouxfx{F}", name=f"iouxfx{F}") for F in range(8)]
            # c/h state for the subtree, cols = local heap index 0..1022
            C = [consts.tile([128, 1024], F32, tag=f"C{h}", name=f"C{h}") for h in range(2)]
            H = [consts.tile([128, 1024], BF16, tag=f"H{h}", name=f"H{h}") for h in range(2)]

            # ---- phase 1: x-projections  IOUXFX = Wc @ x ----
            psum_tags = ["iou0", "iou1", "iou2", "fps"]
            pre_i = 0
            if True:
                for F in range(8):
                    for c0, cw in ((512, 512), (0, 512)):
                        ps = spsum.tile([128, 512], F32, tag=psum_tags[pre_i % 4], bufs=2, name=f"pre_ps{pre_i}")
                        pre_i += 1
                        for kt in range(2):
                            nc.tensor.matmul(
                                ps[:, :cw],
                                sb_wcT[kt][:, 128 * F : 128 * (F + 1)],
                                sb_xT[kt][:, c0 : c0 + cw],
                                start=(kt == 0),
                                stop=(kt == 1),
                            )
                        if pre_i % 2 == 0:
                            nc.vector.tensor_copy(IOUXFX[F][:, c0 : c0 + cw], ps[:, :cw])
                        else:
                            nc.scalar.copy(IOUXFX[F][:, c0 : c0 + cw], ps[:, :cw])

            # ---- phase 2: leaves (local heap 511..1022 -> cols [511:1023)) ----
            a, b = 512, 1024
            for h in range(2):
                sig_i = tmps.tile([128, 512], F32, tag="sig_i")
                nc.scalar.activation(
                    sig_i[:], IOUXFX[0 + h][:, a:b], AFT.Sigmoid,
                    bias=sb_biou_l[:, 0 + h : 1 + h],
                )
                sig_o = tmps.tile([128, 512], F32, tag="sig_o")
                nc.scalar.activation(
                    sig_o[:], IOUXFX[2 + h][:, a:b], AFT.Sigmoid,
                    bias=sb_biou_l[:, 2 + h : 3 + h],
                )
                tanh_u = tmps.tile([128, 512], F32, tag="tanh_u")
                nc.scalar.activation(
                    tanh_u[:], IOUXFX[4 + h][:, a:b], AFT.Tanh,
                    bias=sb_biou_l[:, 4 + h : 5 + h],
                )
                fc = tmps.tile([128, 512], F32, tag="fc")
                nc.scalar.activation(
                    fc[:], IOUXFX[6 + h][:, a:b], AFT.Sigmoid,
                    bias=sb_bf_l[:, h : h + 1],
                )
                iu = tmps.tile([128, 512], F32, tag="iu")
                nc.vector.tensor_mul(iu[:], sig_i[:], tanh_u[:])
                nc.vector.tensor_add(C[h][:, a:b], iu[:], fc[:])
                tanh_c = tmps.tile([128, 512], F32, tag="tanh_c")
                nc.scalar.activation(tanh_c[:], C[h][:, a:b], AFT.Tanh)
                nc.vector.tensor_mul(H[h][:, a:b], sig_o[:], tanh_c[:])

            # ---- internal level routine (feature-major) ----
            def internal_level(n, ioux_lo, childC, childH, Cout, Hout):
                # childC/childH: per h/kt APs [128, 2n] (child cols, heap order)
                # Cout/Hout: per h APs [128, n]
                hs = []
                for kt in range(2):
                    t = tmps.tile([128, max(n, 1)], BF16, tag="hs")
                    nc.vector.tensor_add(
                        t[:, :n], childH[kt][:, 0::2], childH[kt][:, 1::2]
                    )
                    hs.append(t)
                ps_iou = []
                for F in range(6):
                    ps = spsum.tile([128, 512], F32, tag=f"iou{F % 3}", bufs=2, name=f"ps_iou{F}_{n}_{ioux_lo}")
                    for kt in range(2):
                        nc.tensor.matmul(
                            ps[:, :n],
                            sb_wiouhT[kt][:, 128 * F : 128 * (F + 1)],
                            hs[kt][:, :n],
                            start=(kt == 0),
                            stop=(kt == 1),
                        )
                    pre = tmps.tile([128, max(n, 1)], F32, tag=f"ioupre{F}", name=f"ioupre{F}_{n}_{ioux_lo}")
                    nc.vector.tensor_add(
                        pre[:, :n], ps[:, :n], IOUXFX[F][:, ioux_lo : ioux_lo + n]
                    )
                    ps_iou.append(pre)
                ps_f = []
                for h in range(2):
                    ps = spsum.tile([128, 512], F32, tag="fps", bufs=2, name=f"ps_f{h}_{n}_{ioux_lo}")
                    for kt in range(2):
                        nc.tensor.matmul(
                            ps[:, : 2 * n],
                            sb_wfhT[kt][:, 128 * h : 128 * (h + 1)],
                            childH[kt],
                            start=(kt == 0),
                            stop=(kt == 1),
                        )
                    # + fx[parent] duplicated onto both child slots
                    fxdup = (
                        IOUXFX[6 + h][:, ioux_lo : ioux_lo + n]
                        .unsqueeze(2)
                        .broadcast_to([128, n, 2])
                    )
                    fpre = tmps.tile([128, max(2 * n, 1)], F32, tag=f"fpre{h}", name=f"fpre{h}_{n}_{ioux_lo}")
                    nc.vector.tensor_add(
                        fpre[:, : 2 * n].rearrange("p (n two) -> p n two", two=2),
                        ps[:, : 2 * n].rearrange("p (n two) -> p n two", two=2),
                        fxdup,
                    )
                    ps_f.append(fpre)
                for h in range(2):
                    sig_i = tmps.tile([128, max(n, 1)], F32, tag="sig_i")
                    nc.scalar.activation(
                        sig_i[:, :n], ps_iou[0 + h][:, :n], AFT.Sigmoid,
                        bias=sb_biou_i[:, 0 + h : 1 + h],
                    )
                    sig_o = tmps.tile([128, max(n, 1)], F32, tag="sig_o")
                    nc.scalar.activation(
                        sig_o[:, :n], ps_iou[2 + h][:, :n], AFT.Sigmoid,
                        bias=sb_biou_i[:, 2 + h : 3 + h],
                    )
                    tanh_u = tmps.tile([128, max(n, 1)], F32, tag="tanh_u")
                    nc.scalar.activation(
                        tanh_u[:, :n], ps_iou[4 + h][:, :n], AFT.Tanh,
                        bias=sb_biou_i[:, 4 + h : 5 + h],
                    )
                    f = tmps.tile([128, max(2 * n, 1)], F32, tag="f")
                    nc.scalar.activation(
                        f[:, : 2 * n], ps_f[h][:, : 2 * n], AFT.Sigmoid,
                        bias=sb_bf_i[:, h : h + 1],
                    )
                    g = tmps.tile([128, max(2 * n, 1)], F32, tag="g")
                    nc.vector.tensor_mul(g[:, : 2 * n], f[:, : 2 * n], childC[h])
                    fc = tmps.tile([128, max(n, 1)], F32, tag="fc")
                    nc.vector.tensor_add(fc[:, :n], g[:, 0 : 2 * n : 2], g[:, 1 : 2 * n : 2])
                    iu = tmps.tile([128, max(n, 1)], F32, tag="iu")
                    nc.vector.tensor_mul(iu[:, :n], sig_i[:, :n], tanh_u[:, :n])
                    nc.vector.tensor_add(Cout[h], iu[:, :n], fc[:, :n])
                    tanh_c = tmps.tile([128, max(n, 1)], F32, tag="tanh_c")
                    nc.scalar.activation(tanh_c[:, :n], Cout[h], AFT.Tanh)
                    nc.vector.tensor_mul(Hout[h], sig_o[:, :n], tanh_c[:, :n])

            # ---- phase 3: subtree internal levels (l = 8 .. 0) ----
            for l in range(8, 2, -1):
                n = 1 << l
                a, b = n, 2 * n
                a2, b2 = 2 * n, 4 * n
                internal_level(
                    n,
                    a,
                    [C[h][:, a2:b2] for h in range(2)],
                    [H[kt][:, a2:b2] for kt in range(2)],
                    [C[h][:, a:b] for h in range(2)],
                    [H[h][:, a:b] for h in range(2)],
                )

            # ---- phase 4: emit the level-3 boundary (8 nodes); rest on host ----
            for h in range(2):
                nc.sync.dma_start(
                    out=out[128 * h : 128 * (h + 1), :], in_=C[h][:, 8:16]
                )
                hroot32 = tmps.tile([128, 8], F32, tag=f"hroot32_{h}", name=f"hroot32_{h}")
                nc.vector.tensor_copy(hroot32[:], H[h][:, 8:16])
                nc.sync.dma_start(
                    out=out[256 + 128 * h : 256 + 128 * (h + 1), :], in_=hroot32[:]
                )
    _split_excess_waits(nc)
    return nc


_NC_CACHE = None


def _get_module():
    global _NC_CACHE
    if _NC_CACHE is None:
        _NC_CACHE = _build_module()
    return _NC_CACHE


def _expected_children():
    j = (N_NODES - 1) - np.arange(N_NODES)
    internal = (2 * j + 1) < N_NODES
    ch0 = (N_NODES - 1) - (2 * j + 1)
    ch1 = (N_NODES - 1) - (2 * j + 2)
    children = np.stack(
        [np.where(internal, ch0, 0), np.where(internal, ch1, 0)], axis=1
    ).astype(np.int32)
    mask = np.stack([internal, internal], axis=1)
    return children, mask


def _reference_numpy(emb, W_ioux, b_ioux, W_iouh, b_iouh, W_fx, b_fx, W_fh, b_fh,
                     ops, children, child_mask):
    # generic fallback (matches reference.py) for unexpected tree structure
    def sigmoid(v):
        return 1.0 / (1.0 + np.exp(-v))

    N = ops.shape[0]
    Md = W_fh.shape[0]
    x = emb[ops]
    iou_x = x @ W_ioux.T + b_ioux
    fx_all = x @ W_fx.T + b_fx
    ones = np.ones((Md,), np.float32)
    leaf_fh = ones @ W_fh.T + b_fh
    maskf = child_mask.astype(np.float32)
    c_arr = np.zeros((N, Md), np.float32)
    h_arr = np.zeros((N, Md), np.float32)
    for t in range(N):
        idx = children[t]
        m = maskf[t][:, None]
        ch_c = c_arr[idx] * m
        ch_h = h_arr[idx] * m
        is_leaf = maskf[t].sum() == 0
        h_sum = ones if is_leaf else ch_h.sum(0)
        iou = iou_x[t] + h_sum @ W_iouh.T + b_iouh
        i, o, u = np.split(iou, 3)
        i, o, u = sigmoid(i), sigmoid(o), np.tanh(u)
        f = sigmoid(ch_h @ W_fh.T + b_fh + fx_all[t])
        fc_int = (f * ch_c).sum(0)
        fc_leaf = sigmoid(leaf_fh + fx_all[t])
        fc = fc_leaf if is_leaf else fc_int
        c = i * u + fc
        h = o * np.tanh(c)
        c_arr[t] = c
        h_arr[t] = h
    return np.stack([c_arr[N - 1], h_arr[N - 1]])


def _col_index_for_core(k):
    # col 0 pad; cols 1..1023: subtree-local heap order shifted by +1
    # (level l at cols [2^l, 2^(l+1)), leaves exactly at [512, 1024))
    idx = np.zeros(NCOL, np.int64)
    for l in range(SUB_LEVELS):
        n = 1 << l
        g0 = (1 << (3 + l)) - 1 + k * n
        idx[n : 2 * n] = g0 + np.arange(n)
    return idx


def kernel(**inputs):
    emb = np.asarray(inputs["emb"], np.float32)
    W_ioux = np.asarray(inputs["W_ioux"], np.float32)
    b_ioux = np.asarray(inputs["b_ioux"], np.float32)
    W_iouh = np.asarray(inputs["W_iouh"], np.float32)
    b_iouh = np.asarray(inputs["b_iouh"], np.float32)
    W_fx = np.asarray(inputs["W_fx"], np.float32)
    b_fx = np.asarray(inputs["b_fx"], np.float32)
    W_fh = np.asarray(inputs["W_fh"], np.float32)
    b_fh = np.asarray(inputs["b_fh"], np.float32)
    ops = np.asarray(inputs["ops"], np.int32)
    children = np.asarray(inputs["children"], np.int32)
    child_mask = np.asarray(inputs["child_mask"])

    exp_children, exp_mask = _expected_children()
    if (
        ops.shape[0] != N_NODES
        or not np.array_equal(children, exp_children)
        or not np.array_equal(child_mask.astype(bool), exp_mask)
    ):
        return _reference_numpy(
            emb, W_ioux, b_ioux, W_iouh, b_iouh, W_fx, b_fx, W_fh, b_fh,
            ops, children, child_mask,
        )

    # ---- host prep ----
    x = emb[ops]  # [8191, 256]
    x_heap = x[::-1]  # heap order: topo t = N-1-j
    import ml_dtypes

    bf16 = ml_dtypes.bfloat16
    wcT = np.ascontiguousarray(np.concatenate([W_ioux, W_fx], 0).T).astype(bf16)
    wiouhT = np.ascontiguousarray(W_iouh.T).astype(bf16)
    wfhT = np.ascontiguousarray(W_fh.T).astype(bf16)
    b_iou_int = np.ascontiguousarray((b_ioux + b_iouh).reshape(6, 128).T)
    b_iou_leaf = np.ascontiguousarray(
        (b_ioux + W_iouh.sum(1) + b_iouh).reshape(6, 128).T
    )
    b_f_int = np.ascontiguousarray((b_fh + b_fx).reshape(2, 128).T)
    b_f_leaf = np.ascontiguousarray((W_fh.sum(1) + b_fh + b_fx).reshape(2, 128).T)

    common = {
        "wcT": wcT,
        "wiouhT": wiouhT,
        "wfhT": wfhT,
        "b_iou_int": b_iou_int,
        "b_iou_leaf": b_iou_leaf,
        "b_f_int": b_f_int,
        "b_f_leaf": b_f_leaf,
    }
    in_maps = []
    for k in range(8):
        idx = _col_index_for_core(k)
        xT = np.ascontiguousarray(x_heap[idx].T.astype(bf16))
        in_maps.append({"xT": xT, **common})

    global _LAST_IN_MAPS
    _LAST_IN_MAPS = in_maps
    nc = _get_module()
    res = run_bass_kernel_spmd(nc, in_maps, list(range(8)))

    # ---- host: subtree levels 2..0 (63 tiny nodes) + global top 7 ----
    def sigmoid(v):
        return 1.0 / (1.0 + np.exp(-v))

    x_top = x_heap[0:63].astype(np.float32)
    iou_x63 = x_top @ W_ioux.T + b_ioux
    fx63 = x_top @ W_fx.T + b_fx

    def cell(iou_x_j, fx_j, hs2, cs2):
        h_sum = hs2[0] + hs2[1]
        iou = iou_x_j + h_sum @ W_iouh.T + b_iouh
        i_g, o_g, u_g = np.split(iou, 3)
        i_g, o_g, u_g = sigmoid(i_g), sigmoid(o_g), np.tanh(u_g)
        f = sigmoid(hs2 @ W_fh.T + b_fh + fx_j)
        fc = (f * cs2).sum(0)
        c = i_g * u_g + fc
        return c, o_g * np.tanh(c)

    c_arr = np.zeros((15, M), np.float32)
    h_arr = np.zeros((15, M), np.float32)
    for k in range(8):
        r = res.results[k]["out"]  # [512, 8], cols = local heap 7..14
        c_loc = np.zeros((15, M), np.float32)
        h_loc = np.zeros((15, M), np.float32)
        c_loc[7:15] = r[0:256].T
        h_loc[7:15] = r[256:512].T
        for j in range(6, -1, -1):
            lvl = j.bit_length() if j else 0  # floor(log2(j+1)): 0,1,1,2,2,2,2
            lvl = int(np.log2(j + 1))
            m = j - ((1 << lvl) - 1)
            g = (1 << (3 + lvl)) - 1 + k * (1 << lvl) + m
            ch = [2 * j + 1, 2 * j + 2]
            c_loc[j], h_loc[j] = cell(
                iou_x63[g], fx63[g],
                h_loc[ch], c_loc[ch],
            )
        c_arr[7 + k] = c_loc[0]
        h_arr[7 + k] = h_loc[0]
    for j in range(6, -1, -1):
        ch = [2 * j + 1, 2 * j + 2]
        c_arr[j], h_arr[j] = cell(
            iou_x63[j], fx63[j], h_arr[ch], c_arr[ch]
        )
    return np.stack([c_arr[0], h_arr[0]]).astype(np.float32)


_LAST_IN_MAPS = None



# revision 7
# speedup vs baseline: 1.2919x; 1.2919x over previous
"""ChildSumTreeLSTM on a complete binary tree (N=8191), 8-core Trainium2.

v2: heap-ordered tree = 7 top nodes + 8 independent 1023-node subtrees,
one per NeuronCore. Per core, feature-major [256 feats x cols] layout with
col = subtree-local heap index (level l at cols [2^l, 2^(l+1)), leaves at
[512,1024)).

Key structure (vs v1):
- x-projections run in fp8e4m3 DoubleRow matmuls (K=256 in one instr,
  0.5 cyc/col) and are computed JUST-IN-TIME into the same PSUM
  accumulation group as each level's h-matmuls (scan weights pre-scaled
  x4096 in bf16 to match the fp8 scaling), so no SBUF x-proj tiles, no
  evacuation copies, no pre-add vector ops.
- f-gate x-terms use a host-duplicated fp8 x tensor (x8d[c] = x8[c//2]) so
  fx[parent] lands directly on both child columns of the f PSUM.
- All sigmoid/tanh run on the ACT engine with bias ports + 1/4096 descale;
  elementwise cell ops split across DVE and GpSimd.
- Device computes leaves + levels 256/128/64; the tiny top levels
  (<=32 per subtree, 511 nodes + global top 7) finish on host.
"""

import numpy as np

import concourse.bass as bass
import concourse.tile as tile
from concourse import mybir
from concourse.bass_utils import run_bass_kernel_spmd

F32 = mybir.dt.float32
BF16 = mybir.dt.bfloat16
FP8 = mybir.dt.float8e4
AFT = mybir.ActivationFunctionType
DR = mybir.MatmulPerfMode.DoubleRow

N_NODES = 8191
D = 256
M = 256
NCOL = 1024
SUB_LEVELS = 10
DESCALE = 1.0 / 4096.0  # x8 = 128*x, wc8 = 32*Wc, wsc = 4096*W


def _split_excess_waits(nc, max_waits=1):
    """walrus in this container allows only 1 sync-wait per instruction.

    Tile can attach several; hoist the extras onto injected same-engine NOPs
    immediately preceding the instruction (same blocking semantics)."""
    k = 0
    for f in nc.m.functions:
        for bb in f.blocks:
            out = []
            changed = False
            for ins in bb.instructions:
                si = ins.sync_info
                w = list(si.on_wait) if si and si.on_wait else []
                if len(w) > max_waits:
                    hoist, keep = w[:-max_waits], w[-max_waits:]
                    for sw in hoist:
                        nop = mybir.InstNoOp(name=f"whoist{k}", ins=[], outs=[])
                        k += 1
                        nop.engine = ins.engine
                        nop.sync_info = mybir.SyncInfo(on_wait=[sw], on_update=[])
                        out.append(nop)
                    si.on_wait = keep
                    changed = True
                out.append(ins)
            if changed:
                bb.instructions = out


def _build_module():
    nc = bass.Bass(num_devices=8)

    # fp8 pack: [:,0]=wc8 (32*Wc feature-major), [:,1]=x8 (128*x), [:,2]=x8d
    fp8pack = nc.dram_tensor("fp8pack", [128, 3, 2, NCOL], FP8, kind="ExternalInput")
    wsc = nc.dram_tensor("wsc", [128, 2, NCOL], BF16, kind="ExternalInput")
    biasd = nc.dram_tensor("biasd", [128, 16], F32, kind="ExternalInput")
    out_c = nc.dram_tensor("out_c", [128, 128], F32, kind="ExternalOutput")
    out_h = nc.dram_tensor("out_h", [128, 128], BF16, kind="ExternalOutput")

    with tile.TileContext(nc) as tc:
        with (
            tc.tile_pool(name="consts", bufs=1) as consts,
            tc.tile_pool(name="tmps", bufs=2) as tmps,
            tc.tile_pool(name="spsum", bufs=1, space="PSUM") as spsum,
        ):
            sb_f8 = consts.tile([128, 3, 2, NCOL], FP8, tag="f8")
            nc.sync.dma_start(out=sb_f8[:], in_=fp8pack[:])
            sb_wsc = consts.tile([128, 2, NCOL], BF16, tag="wsc")
            nc.gpsimd.dma_start(out=sb_wsc[:], in_=wsc[:])
            sb_b = consts.tile([128, 16], F32, tag="bias")
            nc.gpsimd.dma_start(out=sb_b[:], in_=biasd[:])

            wc8 = sb_f8[:, 0]   # [128, 2, 1024] feature-major weights
            x8 = sb_f8[:, 1]    # [128, 2, 1024] node cols
            x8d = sb_f8[:, 2]   # x8d[:, :, c] = x8[:, :, c//2]

            H = consts.tile([128, 2, NCOL], BF16, tag="H")
            C = consts.tile([128, 2, NCOL], F32, tag="C")

            # leaf gate tiles (cols 512..1023)
            l_si = consts.tile([128, 2, 512], BF16, tag="l_si")
            l_so = consts.tile([128, 2, 512], BF16, tag="l_so")
            l_tu = consts.tile([128, 2, 512], BF16, tag="l_tu")
            l_fc = consts.tile([128, 2, 512], F32, tag="l_fc")
            l_iu = consts.tile([128, 2, 512], BF16, tag="l_iu")
            l_tc = consts.tile([128, 2, 512], BF16, tag="l_tc")

            def ps_iou_tile(tag, name):
                return spsum.tile([128, 2, 256], F32, tag=tag, bufs=2, name=name)

            def ps_f_tile(name):
                return spsum.tile([128, 512], F32, tag="pf", bufs=2, name=name)

            # ---- leaf phase: one fp8 DoubleRow MM per F block, acts from PSUM
            # F: 0,1=i  2,3=o  4,5=u  6,7=fx   (j = F%2 feature half)
            leaf_meta = [
                (0, l_si, AFT.Sigmoid, 6), (1, l_si, AFT.Sigmoid, 7),
                (2, l_so, AFT.Sigmoid, 8), (3, l_so, AFT.Sigmoid, 9),
                (4, l_tu, AFT.Tanh, 10), (5, l_tu, AFT.Tanh, 11),
                (6, l_fc, AFT.Sigmoid, 14), (7, l_fc, AFT.Sigmoid, 15),
            ]
            # j=0 gates first so the j=0 cell chain starts early
            for F, gate, func, bcol in [leaf_meta[i] for i in (0, 2, 4, 6, 1, 3, 5, 7)]:
                if F < 6:
                    ps = ps_iou_tile(["pi", "pi", "po", "po", "pu", "pu"][F], f"lps{F}")
                    ps_ap = ps[:, :, :]
                else:
                    ps = ps_f_tile(f"lps{F}")
                    ps_ap = ps[:, :]
                nc.tensor.matmul(
                    ps_ap, wc8[:, :, 128 * F : 128 * (F + 1)], x8[:, :, 512:1024],
                    start=True, stop=True, perf_mode=DR,
                )
                nc.scalar.activation(
                    gate[:, F % 2, :], ps_ap, func,
                    bias=sb_b[:, bcol : bcol + 1], scale=DESCALE,
                )

            # leaf cell: iu, c (split engines), tanh_c, h
            nc.vector.tensor_mul(l_iu[:], l_si[:], l_tu[:])
            nc.vector.tensor_add(C[:, 0, 512:1024], l_iu[:, 0], l_fc[:, 0])
            nc.gpsimd.tensor_add(C[:, 1, 512:1024], l_iu[:, 1], l_fc[:, 1])
            nc.scalar.activation(l_tc[:, 0, :], C[:, 0, 512:1024], AFT.Tanh)
            nc.scalar.activation(l_tc[:, 1, :], C[:, 1, 512:1024], AFT.Tanh)
            nc.vector.tensor_mul(H[:, :, 512:1024], l_so[:], l_tc[:])

            # ---- internal levels n = 256, 128, 64 ----
            for n in (256, 128, 64):
                a, b2 = n, 2 * n          # parent cols
                ca, cb = 2 * n, 4 * n     # child cols

                hs = tmps.tile([128, 2, 256], BF16, tag="hs")
                nc.vector.tensor_add(
                    hs[:, :, :n], H[:, :, ca:cb:2], H[:, :, ca + 1 : cb : 2]
                )

                # f path first (longer chain): JIT fx + 2 h-matmuls per half
                ps_f = []
                for h in range(2):
                    ps = ps_f_tile(f"psf{h}_{n}")
                    Fb = 768 + 128 * h
                    nc.tensor.matmul(
                        ps[:, : 2 * n], wc8[:, :, Fb : Fb + 128], x8d[:, :, ca:cb],
                        start=True, stop=False, perf_mode=DR,
                    )
                    for j in range(2):
                        nc.tensor.matmul(
                            ps[:, : 2 * n],
                            sb_wsc[:, j, Fb : Fb + 128],
                            H[:, j, ca:cb],
                            start=False, stop=(j == 1),
                        )
                    ps_f.append(ps)

                # iou: 3 pair tiles (i,o,u), JIT + 2 h-matmuls per F
                ps_iou = []
                for pair in range(3):
                    ps = ps_iou_tile(["pi", "po", "pu"][pair], f"ps{'iou'[pair]}_{n}")
                    for sub in range(2):
                        F = 2 * pair + sub
                        nc.tensor.matmul(
                            ps[:, sub, :n],
                            wc8[:, :, 128 * F : 128 * (F + 1)],
                            x8[:, :, a:b2],
                            start=True, stop=False, perf_mode=DR,
                        )
                        for j in range(2):
                            nc.tensor.matmul(
                                ps[:, sub, :n],
                                sb_wsc[:, j, 128 * F : 128 * (F + 1)],
                                hs[:, j, :n],
                                start=False, stop=(j == 1),
                            )
                    ps_iou.append(ps)

                # acts
                t_f = tmps.tile([128, 2, 512], BF16, tag="t_f")
                for h in range(2):
                    nc.scalar.activation(
                        t_f[:, h, : 2 * n], ps_f[h][:, : 2 * n], AFT.Sigmoid,
                        bias=sb_b[:, 12 + h : 13 + h], scale=DESCALE,
                    )
                t_si = tmps.tile([128, 2, 256], BF16, tag="t_si")
                t_so = tmps.tile([128, 2, 256], BF16, tag="t_so")
                t_tu = tmps.tile([128, 2, 256], BF16, tag="t_tu")
                for pair, gate, func in (
                    (0, t_si, AFT.Sigmoid), (1, t_so, AFT.Sigmoid), (2, t_tu, AFT.Tanh),
                ):
                    for sub in range(2):
                        F = 2 * pair + sub
                        nc.scalar.activation(
                            gate[:, sub, :n], ps_iou[pair][:, sub, :n], func,
                            bias=sb_b[:, F : F + 1], scale=DESCALE,
                        )

                # cell
                g = tmps.tile([128, 2, 512], F32, tag="g")
                fc = tmps.tile([128, 2, 256], F32, tag="fc")
                nc.vector.tensor_mul(g[:, 0, : 2 * n], t_f[:, 0, : 2 * n], C[:, 0, ca:cb])
                nc.gpsimd.tensor_mul(g[:, 1, : 2 * n], t_f[:, 1, : 2 * n], C[:, 1, ca:cb])
                nc.vector.tensor_add(
                    fc[:, 0, :n], g[:, 0, 0 : 2 * n : 2], g[:, 0, 1 : 2 * n : 2]
                )
                nc.gpsimd.tensor_add(
                    fc[:, 1, :n], g[:, 1, 0 : 2 * n : 2], g[:, 1, 1 : 2 * n : 2]
                )
                iu = tmps.tile([128, 2, 256], BF16, tag="iu")
                nc.vector.tensor_mul(iu[:, :, :n], t_si[:, :, :n], t_tu[:, :, :n])
                nc.vector.tensor_add(C[:, :, a:b2], iu[:, :, :n], fc[:, :, :n])
                t_tc = tmps.tile([128, 2, 256], BF16, tag="t_tc")
                nc.scalar.activation(t_tc[:, :, :n], C[:, :, a:b2], AFT.Tanh)
                nc.vector.tensor_mul(H[:, :, a:b2], t_so[:, :, :n], t_tc[:, :, :n])

            # ---- emit level-64 boundary ----
            nc.sync.dma_start(out=out_c[:, :], in_=C[:, :, 64:128])
            nc.sync.dma_start(out=out_h[:, :], in_=H[:, :, 64:128])

    _split_excess_waits(nc)
    return nc


_NC_CACHE = None


def _get_module():
    global _NC_CACHE
    if _NC_CACHE is None:
        _NC_CACHE = _build_module()
    return _NC_CACHE


def _expected_children():
    j = (N_NODES - 1) - np.arange(N_NODES)
    internal = (2 * j + 1) < N_NODES
    ch0 = (N_NODES - 1) - (2 * j + 1)
    ch1 = (N_NODES - 1) - (2 * j + 2)
    children = np.stack(
        [np.where(internal, ch0, 0), np.where(internal, ch1, 0)], axis=1
    ).astype(np.int32)
    mask = np.stack([internal, internal], axis=1)
    return children, mask


def _reference_numpy(emb, W_ioux, b_ioux, W_iouh, b_iouh, W_fx, b_fx, W_fh, b_fh,
                     ops, children, child_mask):
    # generic fallback (matches reference.py) for unexpected tree structure
    def sigmoid(v):
        return 1.0 / (1.0 + np.exp(-v))

    N = ops.shape[0]
    Md = W_fh.shape[0]
    x = emb[ops]
    iou_x = x @ W_ioux.T + b_ioux
    fx_all = x @ W_fx.T + b_fx
    ones = np.ones((Md,), np.float32)
    leaf_fh = ones @ W_fh.T + b_fh
    maskf = child_mask.astype(np.float32)
    c_arr = np.zeros((N, Md), np.float32)
    h_arr = np.zeros((N, Md), np.float32)
    for t in range(N):
        idx = children[t]
        m = maskf[t][:, None]
        ch_c = c_arr[idx] * m
        ch_h = h_arr[idx] * m
        is_leaf = maskf[t].sum() == 0
        h_sum = ones if is_leaf else ch_h.sum(0)
        iou = iou_x[t] + h_sum @ W_iouh.T + b_iouh
        i, o, u = np.split(iou, 3)
        i, o, u = sigmoid(i), sigmoid(o), np.tanh(u)
        f = sigmoid(ch_h @ W_fh.T + b_fh + fx_all[t])
        fc_int = (f * ch_c).sum(0)
        fc_leaf = sigmoid(leaf_fh + fx_all[t])
        fc = fc_leaf if is_leaf else fc_int
        c = i * u + fc
        h = o * np.tanh(c)
        c_arr[t] = c
        h_arr[t] = h
    return np.stack([c_arr[N - 1], h_arr[N - 1]])


def _col_index_for_core(k):
    # col 0 pad; cols 1..1023: subtree-local heap order shifted by +1
    # (level l at cols [2^l, 2^(l+1)), leaves exactly at [512, 1024))
    idx = np.zeros(NCOL, np.int64)
    for l in range(SUB_LEVELS):
        n = 1 << l
        g0 = (1 << (3 + l)) - 1 + k * n
        idx[n : 2 * n] = g0 + np.arange(n)
    return idx


def _pack_fm(mat, dtype):
    # mat [cols, 256] -> [128, 2, cols]: out[p, j, c] = mat[c, j*128+p]
    cols = mat.shape[0]
    return np.ascontiguousarray(
        mat.T.reshape(2, 128, cols).transpose(1, 0, 2)
    ).astype(dtype)


def kernel(**inputs):
    emb = np.asarray(inputs["emb"], np.float32)
    W_ioux = np.asarray(inputs["W_ioux"], np.float32)
    b_ioux = np.asarray(inputs["b_ioux"], np.float32)
    W_iouh = np.asarray(inputs["W_iouh"], np.float32)
    b_iouh = np.asarray(inputs["b_iouh"], np.float32)
    W_fx = np.asarray(inputs["W_fx"], np.float32)
    b_fx = np.asarray(inputs["b_fx"], np.float32)
    W_fh = np.asarray(inputs["W_fh"], np.float32)
    b_fh = np.asarray(inputs["b_fh"], np.float32)
    ops = np.asarray(inputs["ops"], np.int32)
    children = np.asarray(inputs["children"], np.int32)
    child_mask = np.asarray(inputs["child_mask"])

    exp_children, exp_mask = _expected_children()
    if (
        ops.shape[0] != N_NODES
        or not np.array_equal(children, exp_children)
        or not np.array_equal(child_mask.astype(bool), exp_mask)
    ):
        return _reference_numpy(
            emb, W_ioux, b_ioux, W_iouh, b_iouh, W_fx, b_fx, W_fh, b_fh,
            ops, children, child_mask,
        )

    import ml_dtypes

    fp8 = ml_dtypes.float8_e4m3
    bf16 = ml_dtypes.bfloat16

    # ---- host prep ----
    x = emb[ops]          # [8191, 256] topo order
    x_heap = x[::-1]      # heap order (topo t = N-1-j)

    Wc = np.concatenate([W_ioux, W_fx], 0)       # [1024, 256]
    Ws = np.concatenate([W_iouh, W_fh], 0)       # [1024, 256]
    wc8 = _pack_fm(32.0 * Wc, fp8)               # [128, 2, 1024]
    wsc = _pack_fm(4096.0 * Ws, bf16)

    bias = np.zeros((128, 16), np.float32)
    bias[:, 0:6] = (b_ioux + b_iouh).reshape(6, 128).T
    bias[:, 6:12] = (b_ioux + W_iouh.sum(1) + b_iouh).reshape(6, 128).T
    bias[:, 12:14] = (b_fx + b_fh).reshape(2, 128).T
    bias[:, 14:16] = (b_fx + W_fh.sum(1) + b_fh).reshape(2, 128).T

    dup_idx = np.arange(NCOL) // 2  # x8d[c] = x8[c//2]

    common = {"wsc": wsc, "biasd": bias}
    in_maps = []
    for k in range(8):
        idx = _col_index_for_core(k)
        xv = x_heap[idx]                          # [1024, 256]
        x8 = _pack_fm(128.0 * xv, fp8)
        x8d = x8[:, :, dup_idx]
        fp8pack = np.ascontiguousarray(np.stack([wc8, x8, x8d], axis=1))
        in_maps.append({"fp8pack": fp8pack, **common})

    global _LAST_IN_MAPS
    _LAST_IN_MAPS = in_maps
    nc = _get_module()
    res = run_bass_kernel_spmd(nc, in_maps, list(range(8)))

    # ---- host: subtree levels 32..1 + global top 7 ----
    def sigmoid(v):
        return 1.0 / (1.0 + np.exp(-v))

    # unpack boundary: [128, 128] -> [64 nodes, 256 feats]
    C_loc = np.zeros((8, 128, M), np.float32)
    H_loc = np.zeros((8, 128, M), np.float32)
    for k in range(8):
        rc = res.results[k]["out_c"]              # [128, 128] f32
        rh = res.results[k]["out_h"].astype(np.float32)
        C_loc[k, 64:128] = rc.reshape(128, 2, 64).transpose(2, 1, 0).reshape(64, M)
        H_loc[k, 64:128] = rh.reshape(128, 2, 64).transpose(2, 1, 0).reshape(64, M)

    # x-projections for host nodes (cols 1..63 per core + global top 7)
    idx_all = np.stack([_col_index_for_core(k)[1:64] for k in range(8)])  # [8, 63]
    x_host = x_heap[idx_all.reshape(-1)].astype(np.float32)               # [504, 256]
    iou_xh = (x_host @ W_ioux.T + b_ioux + b_iouh).reshape(8, 63, 3 * M)
    fx_h = (x_host @ W_fx.T + b_fx + b_fh).reshape(8, 63, M)

    for n in (32, 16, 8, 4, 2, 1):
        ch_h = H_loc[:, 2 * n : 4 * n]            # [8, 2n, 256]
        ch_c = C_loc[:, 2 * n : 4 * n]
        hs = ch_h[:, 0::2] + ch_h[:, 1::2]        # [8, n, 256]
        iou = iou_xh[:, n - 1 : 2 * n - 1] + hs @ W_iouh.T
        i_g = sigmoid(iou[:, :, :M])
        o_g = sigmoid(iou[:, :, M : 2 * M])
        u_g = np.tanh(iou[:, :, 2 * M :])
        fxd = np.repeat(fx_h[:, n - 1 : 2 * n - 1], 2, axis=1)
        f = sigmoid(ch_h @ W_fh.T + fxd)
        gfc = f * ch_c
        fc = gfc[:, 0::2] + gfc[:, 1::2]
        c = i_g * u_g + fc
        C_loc[:, n : 2 * n] = c
        H_loc[:, n : 2 * n] = o_g * np.tanh(c)

    # global top 15: nodes 7..14 are the subtree roots (core k -> 7+k)
    x_top = x_heap[0:7].astype(np.float32)
    iou_x7 = x_top @ W_ioux.T + b_ioux + b_iouh
    fx7 = x_top @ W_fx.T + b_fx + b_fh
    c_arr = np.zeros((15, M), np.float32)
    h_arr = np.zeros((15, M), np.float32)
    c_arr[7:15] = C_loc[:, 1]
    h_arr[7:15] = H_loc[:, 1]
    for j in range(6, -1, -1):
        ch = [2 * j + 1, 2 * j + 2]
        hs2 = h_arr[ch]
        iou = iou_x7[j] + (hs2[0] + hs2[1]) @ W_iouh.T
        i_g, o_g, u_g = np.split(iou, 3)
        i_g, o_g, u_g = sigmoid(i_g), sigmoid(o_g), np.tanh(u_g)
        f = sigmoid(hs2 @ W_fh.T + fx7[j])
        fc = (f * c_arr[ch]).sum(0)
        c_arr[j] = i_g * u_g + fc
        h_arr[j] = o_g * np.tanh(c_arr[j])
    return np.stack([c_arr[0], h_arr[0]]).astype(np.float32)


_LAST_IN_MAPS = None


# revision 12
# speedup vs baseline: 1.7536x; 1.3574x over previous
"""ChildSumTreeLSTM on a complete binary tree (N=8191), 8-core Trainium2.

v3: heap-ordered tree = 7 top nodes + 8 independent 1023-node subtrees,
one per NeuronCore. Per core, feature-major [256 feats x cols] layout with
col = subtree-local heap index (level l at cols [2^l, 2^(l+1)), leaves at
[512,1024)).

- x-projections in fp8e4m3 DoubleRow matmuls (K=256 in one instruction),
  computed just-in-time into the same PSUM accumulation group as each
  level's bf16 h-matmuls (scan weights pre-scaled x4096 to match the fp8
  input scaling; activations descale by 1/4096 and add biases via ports).
- f-gate x-terms use a host-duplicated x tensor (x8d[c] = x8[c//2]).
- Input DMA split so the leaf half arrives first; leaf matmuls start
  immediately.
- Dummy matmuls keep the PE busy through activation windows so it stays
  at the fast p-state.
- Device computes leaves + levels 256/128; the top of each subtree
  (<=64, 1023 nodes total + global top 7) finishes on host, vectorized.
"""

import numpy as np

import concourse.bass as bass
import concourse.tile as tile
from concourse import mybir
from concourse.bass_utils import run_bass_kernel_spmd

F32 = mybir.dt.float32
BF16 = mybir.dt.bfloat16
FP8 = mybir.dt.float8e4
AFT = mybir.ActivationFunctionType
DR = mybir.MatmulPerfMode.DoubleRow

N_NODES = 8191
D = 256
M = 256
NCOL = 1024
SUB_LEVELS = 10
DESCALE = 1.0 / 4096.0  # x8 = 128*x, wc8 = 32*Wc, wsc = 4096*W
DEV_LEVELS = (256, 128)  # internal levels computed on device
BOUND = 128              # boundary level emitted to host


def _split_excess_waits(nc, max_waits=1):
    """walrus in this container allows only 1 sync-wait per instruction.

    Tile can attach several; hoist the extras onto injected same-engine NOPs
    immediately preceding the instruction (same blocking semantics)."""
    k = 0
    for f in nc.m.functions:
        for bb in f.blocks:
            out = []
            changed = False
            for ins in bb.instructions:
                si = ins.sync_info
                w = list(si.on_wait) if si and si.on_wait else []
                if len(w) > max_waits:
                    hoist, keep = w[:-max_waits], w[-max_waits:]
                    for sw in hoist:
                        nop = mybir.InstNoOp(name=f"whoist{k}", ins=[], outs=[])
                        k += 1
                        nop.engine = ins.engine
                        nop.sync_info = mybir.SyncInfo(on_wait=[sw], on_update=[])
                        out.append(nop)
                    si.on_wait = keep
                    changed = True
                out.append(ins)
            if changed:
                bb.instructions = out


def _build_module():
    nc = bass.Bass(num_devices=8)

    # head8: wc8 [0:1024] | x8 leaf half [1024:1536] (leaf cols 512..1023)
    head8 = nc.dram_tensor("head8", [128, 2, 1536], FP8, kind="ExternalInput")
    # rest8: x8 internal cols 0..511 [0:512] | x8d cols 128..1023 [512:1408]
    rest8 = nc.dram_tensor("rest8", [128, 2, 1408], FP8, kind="ExternalInput")
    wsc = nc.dram_tensor("wsc", [128, 2, NCOL], BF16, kind="ExternalInput")
    biasd = nc.dram_tensor("biasd", [128, 16], F32, kind="ExternalInput")
    out_c = nc.dram_tensor("out_c", [128, 2 * BOUND], F32, kind="ExternalOutput")
    out_h = nc.dram_tensor("out_h", [128, 2 * BOUND], BF16, kind="ExternalOutput")

    with tile.TileContext(nc) as tc:
        with (
            tc.tile_pool(name="consts", bufs=1) as consts,
            tc.tile_pool(name="tmps", bufs=2) as tmps,
            tc.tile_pool(name="spsum", bufs=1, space="PSUM") as spsum,
        ):
            sb_h8 = consts.tile([128, 2, 1536], FP8, tag="h8")
            nc.sync.dma_start(out=sb_h8[:], in_=head8[:])
            sb_r8 = consts.tile([128, 2, 1408], FP8, tag="r8")
            nc.sync.dma_start(out=sb_r8[:], in_=rest8[:])
            sb_wsc = consts.tile([128, 2, NCOL], BF16, tag="wsc")
            nc.gpsimd.dma_start(out=sb_wsc[:], in_=wsc[:])
            sb_b = consts.tile([128, 16], F32, tag="bias")
            nc.gpsimd.dma_start(out=sb_b[:], in_=biasd[:])

            wc8 = sb_h8[:, :, 0:1024]
            x8leaf = sb_h8[:, :, 1024:1536]    # leaf cols 512..1023
            x8int = sb_r8[:, :, 0:512]         # cols 0..511

            def x8d_ap(lo, hi):  # duplicated-parent cols lo..hi (128<=lo)
                return sb_r8[:, :, 512 + lo - 128 : 512 + hi - 128]

            H = consts.tile([128, 2, NCOL], BF16, tag="H")
            C = consts.tile([128, 2, NCOL], F32, tag="C")

            # leaf gate tiles (cols 512..1023)
            l_si = consts.tile([128, 2, 512], BF16, tag="l_si")
            l_so = consts.tile([128, 2, 512], BF16, tag="l_so")
            l_tu = consts.tile([128, 2, 512], BF16, tag="l_tu")
            l_fc = consts.tile([128, 2, 512], F32, tag="l_fc")
            l_iu = consts.tile([128, 2, 512], BF16, tag="l_iu")
            l_tc = consts.tile([128, 2, 512], BF16, tag="l_tc")
            hs = consts.tile([128, 2, 256], BF16, tag="hs_l")

            def ps_iou_tile(tag, name):
                return spsum.tile([128, 2, 256], F32, tag=tag, bufs=2, name=name)

            def ps_f_tile(name):
                return spsum.tile([128, 512], F32, tag="pf", bufs=2, name=name)

            def dummy_mms(count, target_ap):
                # PE p-state keepalive: throwaway fp8 MMs into a PSUM region
                # that the next real group resets with start=True.
                for _ in range(count):
                    nc.tensor.matmul(
                        target_ap, wc8[:, :, 0:128], wc8[:, :, 0:512],
                        start=True, stop=True, perf_mode=DR,
                        skip_group_check=True,
                    )

            # ---- leaf phase ----
            # F: 0,1=i  2,3=o  4,5=u  6,7=fx   (sub = F%2 feature half)
            leaf_ps = {}
            order = (0, 4, 6, 2, 1, 5, 7, 3)  # i0,u0,f0,o0, i1,u1,f1,o1
            for F in order:
                if F < 6:
                    ps = ps_iou_tile(["pi", "pi", "po", "po", "pu", "pu"][F], f"lps{F}")
                    ps_ap = ps[:, :, :]
                else:
                    ps = ps_f_tile(f"lps{F}")
                    ps_ap = ps[:, :]
                nc.tensor.matmul(
                    ps_ap, wc8[:, :, 128 * F : 128 * (F + 1)], x8leaf[:],
                    start=True, stop=True, perf_mode=DR,
                )
                leaf_ps[F] = ps_ap
            gate_of = {0: l_si, 1: l_si, 2: l_so, 3: l_so, 4: l_tu, 5: l_tu,
                       6: l_fc, 7: l_fc}
            func_of = {0: AFT.Sigmoid, 1: AFT.Sigmoid, 2: AFT.Sigmoid,
                       3: AFT.Sigmoid, 4: AFT.Tanh, 5: AFT.Tanh,
                       6: AFT.Sigmoid, 7: AFT.Sigmoid}
            bcol_of = {0: 6, 1: 7, 2: 8, 3: 9, 4: 10, 5: 11, 6: 14, 7: 15}

            def leaf_act(F):
                nc.scalar.activation(
                    gate_of[F][:, F % 2, :], leaf_ps[F], func_of[F],
                    bias=sb_b[:, bcol_of[F] : bcol_of[F] + 1], scale=DESCALE,
                )

            # j=0 chain
            for F in (0, 4, 6):
                leaf_act(F)
            nc.vector.tensor_mul(l_iu[:, 0], l_si[:, 0], l_tu[:, 0])
            nc.vector.tensor_add(C[:, 0, 512:1024], l_iu[:, 0], l_fc[:, 0])
            leaf_act(2)
            nc.scalar.activation(l_tc[:, 0, :], C[:, 0, 512:1024], AFT.Tanh)
            nc.vector.tensor_mul(H[:, 0, 512:1024], l_so[:, 0], l_tc[:, 0])
            nc.vector.tensor_add(
                hs[:, 0, :], H[:, 0, 512:1024:2], H[:, 0, 513:1024:2]
            )
            # j=1 chain (adds on gpsimd to keep DVE free)
            for F in (1, 5, 7):
                leaf_act(F)
            nc.vector.tensor_mul(l_iu[:, 1], l_si[:, 1], l_tu[:, 1])
            nc.gpsimd.tensor_add(C[:, 1, 512:1024], l_iu[:, 1], l_fc[:, 1])
            leaf_act(3)
            nc.scalar.activation(l_tc[:, 1, :], C[:, 1, 512:1024], AFT.Tanh)
            nc.vector.tensor_mul(H[:, 1, 512:1024], l_so[:, 1], l_tc[:, 1])
            nc.vector.tensor_add(
                hs[:, 1, :], H[:, 1, 512:1024:2], H[:, 1, 513:1024:2]
            )

            # ---- internal levels ----
            first = True
            for n in DEV_LEVELS:
                a, b2 = n, 2 * n          # parent cols
                ca, cb = 2 * n, 4 * n     # child cols

                if not first:
                    nc.vector.tensor_add(
                        hs[:, 0, :n], H[:, 0, ca:cb:2], H[:, 0, ca + 1 : cb : 2]
                    )
                    nc.vector.tensor_add(
                        hs[:, 1, :n], H[:, 1, ca:cb:2], H[:, 1, ca + 1 : cb : 2]
                    )

                # JIT x-projections (no H dependency: run during prior acts)
                ps_f = []
                for h in range(2):
                    ps = ps_f_tile(f"psf{h}_{n}")
                    Fb = 768 + 128 * h
                    if h == 0:
                        dummy_mms(10 if first else 5, ps[:, 0:512])
                    nc.tensor.matmul(
                        ps[:, : 2 * n], wc8[:, :, Fb : Fb + 128], x8d_ap(ca, cb),
                        start=True, stop=False, perf_mode=DR,
                    )
                    ps_f.append(ps)
                ps_iou = []
                for pair in range(3):
                    ps = ps_iou_tile(["pi", "po", "pu"][pair], f"ps{'iou'[pair]}_{n}")
                    for sub in range(2):
                        F = 2 * pair + sub
                        nc.tensor.matmul(
                            ps[:, sub, :n],
                            wc8[:, :, 128 * F : 128 * (F + 1)],
                            x8int[:, :, a:b2],
                            start=True, stop=False, perf_mode=DR,
                        )
                    ps_iou.append(ps)

                # h-matmuls: all j=0 first, then j=1 (j=0 leaf chain is ready
                # earlier); within j: f, u, i, o
                for j in range(2):
                    last = j == 1
                    for h in range(2):
                        Fb = 768 + 128 * h
                        nc.tensor.matmul(
                            ps_f[h][:, : 2 * n],
                            sb_wsc[:, j, Fb : Fb + 128],
                            H[:, j, ca:cb],
                            start=False, stop=last,
                        )
                    for pair in (2, 0, 1):  # u, i, o
                        for sub in range(2):
                            F = 2 * pair + sub
                            nc.tensor.matmul(
                                ps_iou[pair][:, sub, :n],
                                sb_wsc[:, j, 128 * F : 128 * (F + 1)],
                                hs[:, j, :n],
                                start=False, stop=last,
                            )

                # acts + cell
                t_f = tmps.tile([128, 2, 512], BF16, tag="t_f")
                t_si = tmps.tile([128, 2, 256], BF16, tag="t_si")
                t_so = tmps.tile([128, 2, 256], BF16, tag="t_so")
                t_tu = tmps.tile([128, 2, 256], BF16, tag="t_tu")
                g = tmps.tile([128, 2, 512], F32, tag="g")
                fc = tmps.tile([128, 2, 256], F32, tag="fc")
                iu = tmps.tile([128, 2, 256], BF16, tag="iu")
                t_tc = tmps.tile([128, 2, 256], BF16, tag="t_tc")

                for h in range(2):
                    nc.scalar.activation(
                        t_f[:, h, : 2 * n], ps_f[h][:, : 2 * n], AFT.Sigmoid,
                        bias=sb_b[:, 12 + h : 13 + h], scale=DESCALE,
                    )
                # g/fc: h=0 on DVE, h=1 on gpsimd (off critical path)
                nc.vector.tensor_mul(g[:, 0, : 2 * n], t_f[:, 0, : 2 * n], C[:, 0, ca:cb])
                nc.gpsimd.tensor_mul(g[:, 1, : 2 * n], t_f[:, 1, : 2 * n], C[:, 1, ca:cb])
                nc.vector.tensor_add(
                    fc[:, 0, :n], g[:, 0, 0 : 2 * n : 2], g[:, 0, 1 : 2 * n : 2]
                )
                nc.gpsimd.tensor_add(
                    fc[:, 1, :n], g[:, 1, 0 : 2 * n : 2], g[:, 1, 1 : 2 * n : 2]
                )
                for pair, gate, func in (
                    (2, t_tu, AFT.Tanh), (0, t_si, AFT.Sigmoid), (1, t_so, AFT.Sigmoid),
                ):
                    for sub in range(2):
                        F = 2 * pair + sub
                        nc.scalar.activation(
                            gate[:, sub, :n], ps_iou[pair][:, sub, :n], func,
                            bias=sb_b[:, F : F + 1], scale=DESCALE,
                        )
                nc.vector.tensor_mul(iu[:, :, :n], t_si[:, :, :n], t_tu[:, :, :n])
                nc.vector.tensor_add(C[:, :, a:b2], iu[:, :, :n], fc[:, :, :n])
                nc.scalar.activation(t_tc[:, :, :n], C[:, :, a:b2], AFT.Tanh)
                nc.vector.tensor_mul(H[:, :, a:b2], t_so[:, :, :n], t_tc[:, :, :n])
                first = False

            # ---- emit boundary ----
            nc.sync.dma_start(out=out_c[:, :], in_=C[:, :, BOUND : 2 * BOUND])
            nc.sync.dma_start(out=out_h[:, :], in_=H[:, :, BOUND : 2 * BOUND])

    _split_excess_waits(nc)
    return nc


_NC_CACHE = None


def _get_module():
    global _NC_CACHE
    if _NC_CACHE is None:
        _NC_CACHE = _build_module()
    return _NC_CACHE


def _expected_children():
    j = (N_NODES - 1) - np.arange(N_NODES)
    internal = (2 * j + 1) < N_NODES
    ch0 = (N_NODES - 1) - (2 * j + 1)
    ch1 = (N_NODES - 1) - (2 * j + 2)
    children = np.stack(
        [np.where(internal, ch0, 0), np.where(internal, ch1, 0)], axis=1
    ).astype(np.int32)
    mask = np.stack([internal, internal], axis=1)
    return children, mask


def _reference_numpy(emb, W_ioux, b_ioux, W_iouh, b_iouh, W_fx, b_fx, W_fh, b_fh,
                     ops, children, child_mask):
    # generic fallback (matches reference.py) for unexpected tree structure
    def sigmoid(v):
        return 1.0 / (1.0 + np.exp(-v))

    N = ops.shape[0]
    Md = W_fh.shape[0]
    x = emb[ops]
    iou_x = x @ W_ioux.T + b_ioux
    fx_all = x @ W_fx.T + b_fx
    ones = np.ones((Md,), np.float32)
    leaf_fh = ones @ W_fh.T + b_fh
    maskf = child_mask.astype(np.float32)
    c_arr = np.zeros((N, Md), np.float32)
    h_arr = np.zeros((N, Md), np.float32)
    for t in range(N):
        idx = children[t]
        m = maskf[t][:, None]
        ch_c = c_arr[idx] * m
        ch_h = h_arr[idx] * m
        is_leaf = maskf[t].sum() == 0
        h_sum = ones if is_leaf else ch_h.sum(0)
        iou = iou_x[t] + h_sum @ W_iouh.T + b_iouh
        i, o, u = np.split(iou, 3)
        i, o, u = sigmoid(i), sigmoid(o), np.tanh(u)
        f = sigmoid(ch_h @ W_fh.T + b_fh + fx_all[t])
        fc_int = (f * ch_c).sum(0)
        fc_leaf = sigmoid(leaf_fh + fx_all[t])
        fc = fc_leaf if is_leaf else fc_int
        c = i * u + fc
        h = o * np.tanh(c)
        c_arr[t] = c
        h_arr[t] = h
    return np.stack([c_arr[N - 1], h_arr[N - 1]])


def _col_index_for_core(k):
    # col 0 pad; cols 1..1023: subtree-local heap order shifted by +1
    # (level l at cols [2^l, 2^(l+1)), leaves exactly at [512, 1024))
    idx = np.zeros(NCOL, np.int64)
    for l in range(SUB_LEVELS):
        n = 1 << l
        g0 = (1 << (3 + l)) - 1 + k * n
        idx[n : 2 * n] = g0 + np.arange(n)
    return idx


def _pack_fm(mat, dtype):
    # mat [cols, 256] -> [128, 2, cols]: out[p, j, c] = mat[c, j*128+p]
    cols = mat.shape[0]
    return np.ascontiguousarray(
        mat.T.reshape(2, 128, cols).transpose(1, 0, 2)
    ).astype(dtype)


def kernel(**inputs):
    emb = np.asarray(inputs["emb"], np.float32)
    W_ioux = np.asarray(inputs["W_ioux"], np.float32)
    b_ioux = np.asarray(inputs["b_ioux"], np.float32)
    W_iouh = np.asarray(inputs["W_iouh"], np.float32)
    b_iouh = np.asarray(inputs["b_iouh"], np.float32)
    W_fx = np.asarray(inputs["W_fx"], np.float32)
    b_fx = np.asarray(inputs["b_fx"], np.float32)
    W_fh = np.asarray(inputs["W_fh"], np.float32)
    b_fh = np.asarray(inputs["b_fh"], np.float32)
    ops = np.asarray(inputs["ops"], np.int32)
    children = np.asarray(inputs["children"], np.int32)
    child_mask = np.asarray(inputs["child_mask"])

    exp_children, exp_mask = _expected_children()
    if (
        ops.shape[0] != N_NODES
        or not np.array_equal(children, exp_children)
        or not np.array_equal(child_mask.astype(bool), exp_mask)
    ):
        return _reference_numpy(
            emb, W_ioux, b_ioux, W_iouh, b_iouh, W_fx, b_fx, W_fh, b_fh,
            ops, children, child_mask,
        )

    import ml_dtypes

    fp8 = ml_dtypes.float8_e4m3
    bf16 = ml_dtypes.bfloat16

    # ---- host prep ----
    x = emb[ops]          # [8191, 256] topo order
    x_heap = x[::-1]      # heap order (topo t = N-1-j)

    Wc = np.concatenate([W_ioux, W_fx], 0)       # [1024, 256]
    Ws = np.concatenate([W_iouh, W_fh], 0)       # [1024, 256]
    wc8 = _pack_fm(32.0 * Wc, fp8)               # [128, 2, 1024]
    wsc = _pack_fm(4096.0 * Ws, bf16)

    bias = np.zeros((128, 16), np.float32)
    bias[:, 0:6] = (b_ioux + b_iouh).reshape(6, 128).T
    bias[:, 6:12] = (b_ioux + W_iouh.sum(1) + b_iouh).reshape(6, 128).T
    bias[:, 12:14] = (b_fx + b_fh).reshape(2, 128).T
    bias[:, 14:16] = (b_fx + W_fh.sum(1) + b_fh).reshape(2, 128).T

    common = {"wsc": wsc, "biasd": bias}
    in_maps = []
    for k in range(8):
        idx = _col_index_for_core(k)
        xv = x_heap[idx]                          # [1024, 256]
        x8 = _pack_fm(128.0 * xv, fp8)
        head8 = np.ascontiguousarray(
            np.concatenate([wc8, x8[:, :, 512:1024]], axis=2)
        )
        x8d = x8[:, :, np.arange(128, 1024) // 2]
        rest8 = np.ascontiguousarray(
            np.concatenate([x8[:, :, 0:512], x8d], axis=2)
        )
        in_maps.append({"head8": head8, "rest8": rest8, **common})

    global _LAST_IN_MAPS
    _LAST_IN_MAPS = in_maps
    nc = _get_module()
    res = run_bass_kernel_spmd(nc, in_maps, list(range(8)))

    # ---- host: subtree levels 64..1 + global top 7 ----
    def sigmoid(v):
        return 1.0 / (1.0 + np.exp(-v))

    # unpack boundary: [128, 2*BOUND] -> [BOUND nodes, 256 feats]
    C_loc = np.zeros((8, 2 * BOUND, M), np.float32)
    H_loc = np.zeros((8, 2 * BOUND, M), np.float32)
    for k in range(8):
        rc = res.results[k]["out_c"]
        rh = res.results[k]["out_h"].astype(np.float32)
        C_loc[k, BOUND:] = rc.reshape(128, 2, BOUND).transpose(2, 1, 0).reshape(BOUND, M)
        H_loc[k, BOUND:] = rh.reshape(128, 2, BOUND).transpose(2, 1, 0).reshape(BOUND, M)

    # x-projections for host nodes (cols 1..BOUND-1 per core + global top 7)
    nb = BOUND - 1
    idx_all = np.stack([_col_index_for_core(k)[1:BOUND] for k in range(8)])
    x_host = x_heap[idx_all.reshape(-1)].astype(np.float32)
    iou_xh = (x_host @ W_ioux.T + b_ioux + b_iouh).reshape(8, nb, 3 * M)
    fx_h = (x_host @ W_fx.T + b_fx + b_fh).reshape(8, nb, M)

    n = BOUND // 2
    while n >= 1:
        ch_h = H_loc[:, 2 * n : 4 * n]            # [8, 2n, 256]
        ch_c = C_loc[:, 2 * n : 4 * n]
        hsum = ch_h[:, 0::2] + ch_h[:, 1::2]      # [8, n, 256]
        iou = iou_xh[:, n - 1 : 2 * n - 1] + hsum @ W_iouh.T
        i_g = sigmoid(iou[:, :, :M])
        o_g = sigmoid(iou[:, :, M : 2 * M])
        u_g = np.tanh(iou[:, :, 2 * M :])
        fxd = np.repeat(fx_h[:, n - 1 : 2 * n - 1], 2, axis=1)
        f = sigmoid(ch_h @ W_fh.T + fxd)
        gfc = f * ch_c
        fcs = gfc[:, 0::2] + gfc[:, 1::2]
        c = i_g * u_g + fcs
        C_loc[:, n : 2 * n] = c
        H_loc[:, n : 2 * n] = o_g * np.tanh(c)
        n //= 2

    # global top 15: nodes 7..14 are the subtree roots (core k -> 7+k)
    x_top = x_heap[0:7].astype(np.float32)
    iou_x7 = x_top @ W_ioux.T + b_ioux + b_iouh
    fx7 = x_top @ W_fx.T + b_fx + b_fh
    c_arr = np.zeros((15, M), np.float32)
    h_arr = np.zeros((15, M), np.float32)
    c_arr[7:15] = C_loc[:, 1]
    h_arr[7:15] = H_loc[:, 1]
    for j in range(6, -1, -1):
        ch = [2 * j + 1, 2 * j + 2]
        hs2 = h_arr[ch]
        iou = iou_x7[j] + (hs2[0] + hs2[1]) @ W_iouh.T
        i_g, o_g, u_g = np.split(iou, 3)
        i_g, o_g, u_g = sigmoid(i_g), sigmoid(o_g), np.tanh(u_g)
        f = sigmoid(hs2 @ W_fh.T + fx7[j])
        fcs = (f * c_arr[ch]).sum(0)
        c_arr[j] = i_g * u_g + fcs
        h_arr[j] = o_g * np.tanh(c_arr[j])
    return np.stack([c_arr[0], h_arr[0]]).astype(np.float32)


_LAST_IN_MAPS = None


# revision 13
# speedup vs baseline: 1.8453x; 1.0523x over previous
"""ChildSumTreeLSTM on a complete binary tree (N=8191), 8-core Trainium2.

v3: heap-ordered tree = 7 top nodes + 8 independent 1023-node subtrees,
one per NeuronCore. Per core, feature-major [256 feats x cols] layout with
col = subtree-local heap index (level l at cols [2^l, 2^(l+1)), leaves at
[512,1024)).

- x-projections in fp8e4m3 DoubleRow matmuls (K=256 in one instruction),
  computed just-in-time into the same PSUM accumulation group as each
  level's bf16 h-matmuls (scan weights pre-scaled x4096 to match the fp8
  input scaling; activations descale by 1/4096 and add biases via ports).
- f-gate x-terms use a host-duplicated x tensor (x8d[c] = x8[c//2]).
- Input DMA split so the leaf half arrives first; leaf matmuls start
  immediately.
- Dummy matmuls keep the PE busy through activation windows so it stays
  at the fast p-state.
- Device computes leaves + levels 256/128; the top of each subtree
  (<=64, 1023 nodes total + global top 7) finishes on host, vectorized.
"""

import numpy as np

import concourse.bass as bass
import concourse.tile as tile
from concourse import mybir
from concourse.bass_utils import run_bass_kernel_spmd

F32 = mybir.dt.float32
BF16 = mybir.dt.bfloat16
FP8 = mybir.dt.float8e4
AFT = mybir.ActivationFunctionType
DR = mybir.MatmulPerfMode.DoubleRow

N_NODES = 8191
D = 256
M = 256
NCOL = 1024
SUB_LEVELS = 10
DESCALE = 1.0 / 4096.0  # x8 = 128*x, wc8 = 32*Wc, wsc = 4096*W
DEV_LEVELS = (256, 128)  # internal levels computed on device
BOUND = 128              # boundary level emitted to host


def _split_excess_waits(nc, max_waits=1):
    """walrus in this container allows only 1 sync-wait per instruction.

    Tile can attach several; hoist the extras onto injected same-engine NOPs
    immediately preceding the instruction (same blocking semantics)."""
    k = 0
    for f in nc.m.functions:
        for bb in f.blocks:
            out = []
            changed = False
            for ins in bb.instructions:
                si = ins.sync_info
                w = list(si.on_wait) if si and si.on_wait else []
                if len(w) > max_waits:
                    hoist, keep = w[:-max_waits], w[-max_waits:]
                    for sw in hoist:
                        nop = mybir.InstNoOp(name=f"whoist{k}", ins=[], outs=[])
                        k += 1
                        nop.engine = ins.engine
                        nop.sync_info = mybir.SyncInfo(on_wait=[sw], on_update=[])
                        out.append(nop)
                    si.on_wait = keep
                    changed = True
                out.append(ins)
            if changed:
                bb.instructions = out


def _build_module():
    nc = bass.Bass(num_devices=8)

    # head8: wc8 [0:1024] | x8 leaf half [1024:1536] (leaf cols 512..1023)
    head8 = nc.dram_tensor("head8", [128, 2, 1536], FP8, kind="ExternalInput")
    # rest8: x8 internal cols 0..511 [0:512] | x8d cols 128..1023 [512:1408]
    rest8 = nc.dram_tensor("rest8", [128, 2, 1408], FP8, kind="ExternalInput")
    wsc = nc.dram_tensor("wsc", [128, 2, NCOL], BF16, kind="ExternalInput")
    biasd = nc.dram_tensor("biasd", [128, 16], F32, kind="ExternalInput")
    out_c = nc.dram_tensor("out_c", [128, 2 * BOUND], BF16, kind="ExternalOutput")
    out_h = nc.dram_tensor("out_h", [128, 2 * BOUND], BF16, kind="ExternalOutput")

    with tile.TileContext(nc) as tc:
        with (
            tc.tile_pool(name="consts", bufs=1) as consts,
            tc.tile_pool(name="tmps", bufs=2) as tmps,
            tc.tile_pool(name="spsum", bufs=1, space="PSUM") as spsum,
        ):
            sb_b = consts.tile([128, 16], F32, tag="bias")
            nc.sync.dma_start(out=sb_b[:], in_=biasd[:])
            sb_h8 = consts.tile([128, 2, 1536], FP8, tag="h8")
            nc.sync.dma_start(out=sb_h8[:], in_=head8[:])
            sb_r8 = consts.tile([128, 2, 1408], FP8, tag="r8")
            nc.sync.dma_start(out=sb_r8[:], in_=rest8[:])
            sb_wsc = consts.tile([128, 2, NCOL], BF16, tag="wsc")
            nc.sync.dma_start(out=sb_wsc[:], in_=wsc[:])

            wc8 = sb_h8[:, :, 0:1024]
            x8leaf = sb_h8[:, :, 1024:1536]    # leaf cols 512..1023
            x8int = sb_r8[:, :, 0:512]         # cols 0..511

            def x8d_ap(lo, hi):  # duplicated-parent cols lo..hi (128<=lo)
                return sb_r8[:, :, 512 + lo - 128 : 512 + hi - 128]

            H = consts.tile([128, 2, NCOL], BF16, tag="H")
            C = consts.tile([128, 2, NCOL], BF16, tag="C")

            # leaf gate tiles (cols 512..1023)
            l_si = consts.tile([128, 2, 512], BF16, tag="l_si")
            l_so = consts.tile([128, 2, 512], BF16, tag="l_so")
            l_tu = consts.tile([128, 2, 512], BF16, tag="l_tu")
            l_fc = consts.tile([128, 2, 512], BF16, tag="l_fc")
            l_iu = consts.tile([128, 2, 512], BF16, tag="l_iu")
            l_tc = consts.tile([128, 2, 512], BF16, tag="l_tc")
            hs = consts.tile([128, 2, 256], BF16, tag="hs_l")

            def ps_iou_tile(tag, name):
                return spsum.tile([128, 2, 256], F32, tag=tag, bufs=2, name=name)

            def ps_f_tile(name):
                return spsum.tile([128, 512], F32, tag="pf", bufs=2, name=name)

            def dummy_mms(count, target_ap):
                # PE p-state keepalive: throwaway fp8 MMs into a PSUM region
                # that the next real group resets with start=True.
                for _ in range(count):
                    nc.tensor.matmul(
                        target_ap, wc8[:, :, 0:128], wc8[:, :, 0:512],
                        start=True, stop=True, perf_mode=DR,
                        skip_group_check=True,
                    )

            # ---- leaf phase ----
            # F: 0,1=i  2,3=o  4,5=u  6,7=fx   (sub = F%2 feature half)
            leaf_ps = {}
            order = (0, 4, 6, 2, 1, 5, 7, 3)  # i0,u0,f0,o0, i1,u1,f1,o1
            for F in order:
                if F < 6:
                    ps = ps_iou_tile(["pi", "pi", "po", "po", "pu", "pu"][F], f"lps{F}")
                    ps_ap = ps[:, :, :]
                else:
                    ps = ps_f_tile(f"lps{F}")
                    ps_ap = ps[:, :]
                nc.tensor.matmul(
                    ps_ap, wc8[:, :, 128 * F : 128 * (F + 1)], x8leaf[:],
                    start=True, stop=True, perf_mode=DR,
                )
                leaf_ps[F] = ps_ap
            gate_of = {0: l_si, 1: l_si, 2: l_so, 3: l_so, 4: l_tu, 5: l_tu,
                       6: l_fc, 7: l_fc}
            func_of = {0: AFT.Sigmoid, 1: AFT.Sigmoid, 2: AFT.Sigmoid,
                       3: AFT.Sigmoid, 4: AFT.Tanh, 5: AFT.Tanh,
                       6: AFT.Sigmoid, 7: AFT.Sigmoid}
            bcol_of = {0: 6, 1: 7, 2: 8, 3: 9, 4: 10, 5: 11, 6: 14, 7: 15}

            def leaf_act(F):
                nc.scalar.activation(
                    gate_of[F][:, F % 2, :], leaf_ps[F], func_of[F],
                    bias=sb_b[:, bcol_of[F] : bcol_of[F] + 1], scale=DESCALE,
                )

            # j=0 chain
            for F in (0, 4, 6):
                leaf_act(F)
            nc.vector.tensor_mul(l_iu[:, 0], l_si[:, 0], l_tu[:, 0])
            nc.vector.tensor_add(C[:, 0, 512:1024], l_iu[:, 0], l_fc[:, 0])
            leaf_act(2)
            nc.scalar.activation(l_tc[:, 0, :], C[:, 0, 512:1024], AFT.Tanh)
            nc.vector.tensor_mul(H[:, 0, 512:1024], l_so[:, 0], l_tc[:, 0])
            nc.vector.tensor_add(
                hs[:, 0, :], H[:, 0, 512:1024:2], H[:, 0, 513:1024:2]
            )
            # j=1 chain (adds on gpsimd to keep DVE free)
            for F in (1, 5, 7):
                leaf_act(F)
            nc.vector.tensor_mul(l_iu[:, 1], l_si[:, 1], l_tu[:, 1])
            nc.vector.tensor_add(C[:, 1, 512:1024], l_iu[:, 1], l_fc[:, 1])
            leaf_act(3)
            nc.scalar.activation(l_tc[:, 1, :], C[:, 1, 512:1024], AFT.Tanh)
            nc.vector.tensor_mul(H[:, 1, 512:1024], l_so[:, 1], l_tc[:, 1])
            nc.vector.tensor_add(
                hs[:, 1, :], H[:, 1, 512:1024:2], H[:, 1, 513:1024:2]
            )

            # ---- internal levels ----
            first = True
            for n in DEV_LEVELS:
                a, b2 = n, 2 * n          # parent cols
                ca, cb = 2 * n, 4 * n     # child cols

                if not first:
                    nc.vector.tensor_add(
                        hs[:, 0, :n], H[:, 0, ca:cb:2], H[:, 0, ca + 1 : cb : 2]
                    )
                    nc.vector.tensor_add(
                        hs[:, 1, :n], H[:, 1, ca:cb:2], H[:, 1, ca + 1 : cb : 2]
                    )

                # JIT x-projections (no H dependency: run during prior acts)
                ps_f = []
                for h in range(2):
                    ps = ps_f_tile(f"psf{h}_{n}")
                    Fb = 768 + 128 * h
                    if h == 0:
                        dummy_mms(10, ps[:, 0:512])
                    nc.tensor.matmul(
                        ps[:, : 2 * n], wc8[:, :, Fb : Fb + 128], x8d_ap(ca, cb),
                        start=True, stop=False, perf_mode=DR,
                    )
                    ps_f.append(ps)
                ps_iou = []
                for pair in range(3):
                    ps = ps_iou_tile(["pi", "po", "pu"][pair], f"ps{'iou'[pair]}_{n}")
                    for sub in range(2):
                        F = 2 * pair + sub
                        nc.tensor.matmul(
                            ps[:, sub, :n],
                            wc8[:, :, 128 * F : 128 * (F + 1)],
                            x8int[:, :, a:b2],
                            start=True, stop=False, perf_mode=DR,
                        )
                    ps_iou.append(ps)

                # h-matmuls: all j=0 first, then j=1 (j=0 leaf chain is ready
                # earlier); within j: f, u, i, o
                for j in range(2):
                    last = j == 1
                    for h in range(2):
                        Fb = 768 + 128 * h
                        nc.tensor.matmul(
                            ps_f[h][:, : 2 * n],
                            sb_wsc[:, j, Fb : Fb + 128],
                            H[:, j, ca:cb],
                            start=False, stop=last,
                        )
                    for pair in (2, 0, 1):  # u, i, o
                        for sub in range(2):
                            F = 2 * pair + sub
                            nc.tensor.matmul(
                                ps_iou[pair][:, sub, :n],
                                sb_wsc[:, j, 128 * F : 128 * (F + 1)],
                                hs[:, j, :n],
                                start=False, stop=last,
                            )

                # acts + cell
                t_f = tmps.tile([128, 2, 512], BF16, tag="t_f")
                t_si = tmps.tile([128, 2, 256], BF16, tag="t_si")
                t_so = tmps.tile([128, 2, 256], BF16, tag="t_so")
                t_tu = tmps.tile([128, 2, 256], BF16, tag="t_tu")
                g = tmps.tile([128, 2, 512], BF16, tag="g")
                fc = tmps.tile([128, 2, 256], BF16, tag="fc")
                iu = tmps.tile([128, 2, 256], BF16, tag="iu")
                t_tc = tmps.tile([128, 2, 256], BF16, tag="t_tc")

                for h in range(2):
                    nc.scalar.activation(
                        t_f[:, h, : 2 * n], ps_f[h][:, : 2 * n], AFT.Sigmoid,
                        bias=sb_b[:, 12 + h : 13 + h], scale=DESCALE,
                    )
                # g/fc: h=0 on DVE, h=1 on gpsimd (off critical path)
                nc.vector.tensor_mul(g[:, 0, : 2 * n], t_f[:, 0, : 2 * n], C[:, 0, ca:cb])
                nc.vector.tensor_mul(g[:, 1, : 2 * n], t_f[:, 1, : 2 * n], C[:, 1, ca:cb])
                nc.vector.tensor_add(
                    fc[:, 0, :n], g[:, 0, 0 : 2 * n : 2], g[:, 0, 1 : 2 * n : 2]
                )
                nc.vector.tensor_add(
                    fc[:, 1, :n], g[:, 1, 0 : 2 * n : 2], g[:, 1, 1 : 2 * n : 2]
                )
                for pair, gate, func in (
                    (2, t_tu, AFT.Tanh), (0, t_si, AFT.Sigmoid), (1, t_so, AFT.Sigmoid),
                ):
                    for sub in range(2):
                        F = 2 * pair + sub
                        nc.scalar.activation(
                            gate[:, sub, :n], ps_iou[pair][:, sub, :n], func,
                            bias=sb_b[:, F : F + 1], scale=DESCALE,
                        )
                nc.vector.tensor_mul(iu[:, :, :n], t_si[:, :, :n], t_tu[:, :, :n])
                nc.vector.tensor_add(C[:, :, a:b2], iu[:, :, :n], fc[:, :, :n])
                if n == BOUND:
                    nc.sync.dma_start(out=out_c[:, :], in_=C[:, :, BOUND : 2 * BOUND])
                nc.scalar.activation(t_tc[:, :, :n], C[:, :, a:b2], AFT.Tanh)
                nc.vector.tensor_mul(H[:, :, a:b2], t_so[:, :, :n], t_tc[:, :, :n])
                first = False

            # ---- emit boundary ----
            nc.gpsimd.dma_start(out=out_h[:, :], in_=H[:, :, BOUND : 2 * BOUND])

    _split_excess_waits(nc)
    return nc


_NC_CACHE = None


def _get_module():
    global _NC_CACHE
    if _NC_CACHE is None:
        _NC_CACHE = _build_module()
    return _NC_CACHE


def _expected_children():
    j = (N_NODES - 1) - np.arange(N_NODES)
    internal = (2 * j + 1) < N_NODES
    ch0 = (N_NODES - 1) - (2 * j + 1)
    ch1 = (N_NODES - 1) - (2 * j + 2)
    children = np.stack(
        [np.where(internal, ch0, 0), np.where(internal, ch1, 0)], axis=1
    ).astype(np.int32)
    mask = np.stack([internal, internal], axis=1)
    return children, mask


def _reference_numpy(emb, W_ioux, b_ioux, W_iouh, b_iouh, W_fx, b_fx, W_fh, b_fh,
                     ops, children, child_mask):
    # generic fallback (matches reference.py) for unexpected tree structure
    def sigmoid(v):
        return 1.0 / (1.0 + np.exp(-v))

    N = ops.shape[0]
    Md = W_fh.shape[0]
    x = emb[ops]
    iou_x = x @ W_ioux.T + b_ioux
    fx_all = x @ W_fx.T + b_fx
    ones = np.ones((Md,), np.float32)
    leaf_fh = ones @ W_fh.T + b_fh
    maskf = child_mask.astype(np.float32)
    c_arr = np.zeros((N, Md), np.float32)
    h_arr = np.zeros((N, Md), np.float32)
    for t in range(N):
        idx = children[t]
        m = maskf[t][:, None]
        ch_c = c_arr[idx] * m
        ch_h = h_arr[idx] * m
        is_leaf = maskf[t].sum() == 0
        h_sum = ones if is_leaf else ch_h.sum(0)
        iou = iou_x[t] + h_sum @ W_iouh.T + b_iouh
        i, o, u = np.split(iou, 3)
        i, o, u = sigmoid(i), sigmoid(o), np.tanh(u)
        f = sigmoid(ch_h @ W_fh.T + b_fh + fx_all[t])
        fc_int = (f * ch_c).sum(0)
        fc_leaf = sigmoid(leaf_fh + fx_all[t])
        fc = fc_leaf if is_leaf else fc_int
        c = i * u + fc
        h = o * np.tanh(c)
        c_arr[t] = c
        h_arr[t] = h
    return np.stack([c_arr[N - 1], h_arr[N - 1]])


def _col_index_for_core(k):
    # col 0 pad; cols 1..1023: subtree-local heap order shifted by +1
    # (level l at cols [2^l, 2^(l+1)), leaves exactly at [512, 1024))
    idx = np.zeros(NCOL, np.int64)
    for l in range(SUB_LEVELS):
        n = 1 << l
        g0 = (1 << (3 + l)) - 1 + k * n
        idx[n : 2 * n] = g0 + np.arange(n)
    return idx


def _pack_fm(mat, dtype):
    # mat [cols, 256] -> [128, 2, cols]: out[p, j, c] = mat[c, j*128+p]
    cols = mat.shape[0]
    return np.ascontiguousarray(
        mat.T.reshape(2, 128, cols).transpose(1, 0, 2)
    ).astype(dtype)


def kernel(**inputs):
    emb = np.asarray(inputs["emb"], np.float32)
    W_ioux = np.asarray(inputs["W_ioux"], np.float32)
    b_ioux = np.asarray(inputs["b_ioux"], np.float32)
    W_iouh = np.asarray(inputs["W_iouh"], np.float32)
    b_iouh = np.asarray(inputs["b_iouh"], np.float32)
    W_fx = np.asarray(inputs["W_fx"], np.float32)
    b_fx = np.asarray(inputs["b_fx"], np.float32)
    W_fh = np.asarray(inputs["W_fh"], np.float32)
    b_fh = np.asarray(inputs["b_fh"], np.float32)
    ops = np.asarray(inputs["ops"], np.int32)
    children = np.asarray(inputs["children"], np.int32)
    child_mask = np.asarray(inputs["child_mask"])

    exp_children, exp_mask = _expected_children()
    if (
        ops.shape[0] != N_NODES
        or not np.array_equal(children, exp_children)
        or not np.array_equal(child_mask.astype(bool), exp_mask)
    ):
        return _reference_numpy(
            emb, W_ioux, b_ioux, W_iouh, b_iouh, W_fx, b_fx, W_fh, b_fh,
            ops, children, child_mask,
        )

    import ml_dtypes

    fp8 = ml_dtypes.float8_e4m3
    bf16 = ml_dtypes.bfloat16

    # ---- host prep ----
    x = emb[ops]          # [8191, 256] topo order
    x_heap = x[::-1]      # heap order (topo t = N-1-j)

    Wc = np.concatenate([W_ioux, W_fx], 0)       # [1024, 256]
    Ws = np.concatenate([W_iouh, W_fh], 0)       # [1024, 256]
    wc8 = _pack_fm(32.0 * Wc, fp8)               # [128, 2, 1024]
    wsc = _pack_fm(4096.0 * Ws, bf16)

    bias = np.zeros((128, 16), np.float32)
    bias[:, 0:6] = (b_ioux + b_iouh).reshape(6, 128).T
    bias[:, 6:12] = (b_ioux + W_iouh.sum(1) + b_iouh).reshape(6, 128).T
    bias[:, 12:14] = (b_fx + b_fh).reshape(2, 128).T
    bias[:, 14:16] = (b_fx + W_fh.sum(1) + b_fh).reshape(2, 128).T

    common = {"wsc": wsc, "biasd": bias}
    in_maps = []
    for k in range(8):
        idx = _col_index_for_core(k)
        xv = x_heap[idx]                          # [1024, 256]
        x8 = _pack_fm(128.0 * xv, fp8)
        head8 = np.ascontiguousarray(
            np.concatenate([wc8, x8[:, :, 512:1024]], axis=2)
        )
        x8d = x8[:, :, np.arange(128, 1024) // 2]
        rest8 = np.ascontiguousarray(
            np.concatenate([x8[:, :, 0:512], x8d], axis=2)
        )
        in_maps.append({"head8": head8, "rest8": rest8, **common})

    global _LAST_IN_MAPS
    _LAST_IN_MAPS = in_maps
    nc = _get_module()
    res = run_bass_kernel_spmd(nc, in_maps, list(range(8)))

    # ---- host: subtree levels 64..1 + global top 7 ----
    def sigmoid(v):
        return 1.0 / (1.0 + np.exp(-v))

    # unpack boundary: [128, 2*BOUND] -> [BOUND nodes, 256 feats]
    C_loc = np.zeros((8, 2 * BOUND, M), np.float32)
    H_loc = np.zeros((8, 2 * BOUND, M), np.float32)
    for k in range(8):
        rc = res.results[k]["out_c"]
        rh = res.results[k]["out_h"].astype(np.float32)
        C_loc[k, BOUND:] = rc.reshape(128, 2, BOUND).transpose(2, 1, 0).reshape(BOUND, M)
        H_loc[k, BOUND:] = rh.reshape(128, 2, BOUND).transpose(2, 1, 0).reshape(BOUND, M)

    # x-projections for host nodes (cols 1..BOUND-1 per core + global top 7)
    nb = BOUND - 1
    idx_all = np.stack([_col_index_for_core(k)[1:BOUND] for k in range(8)])
    x_host = x_heap[idx_all.reshape(-1)].astype(np.float32)
    iou_xh = (x_host @ W_ioux.T + b_ioux + b_iouh).reshape(8, nb, 3 * M)
    fx_h = (x_host @ W_fx.T + b_fx + b_fh).reshape(8, nb, M)

    n = BOUND // 2
    while n >= 1:
        ch_h = H_loc[:, 2 * n : 4 * n]            # [8, 2n, 256]
        ch_c = C_loc[:, 2 * n : 4 * n]
        hsum = ch_h[:, 0::2] + ch_h[:, 1::2]      # [8, n, 256]
        iou = iou_xh[:, n - 1 : 2 * n - 1] + hsum @ W_iouh.T
        i_g = sigmoid(iou[:, :, :M])
        o_g = sigmoid(iou[:, :, M : 2 * M])
        u_g = np.tanh(iou[:, :, 2 * M :])
        fxd = np.repeat(fx_h[:, n - 1 : 2 * n - 1], 2, axis=1)
        f = sigmoid(ch_h @ W_fh.T + fxd)
        gfc = f * ch_c
        fcs = gfc[:, 0::2] + gfc[:, 1::2]
        c = i_g * u_g + fcs
        C_loc[:, n : 2 * n] = c
        H_loc[:, n : 2 * n] = o_g * np.tanh(c)
        n //= 2

    # global top 15: nodes 7..14 are the subtree roots (core k -> 7+k)
    x_top = x_heap[0:7].astype(np.float32)
    iou_x7 = x_top @ W_ioux.T + b_ioux + b_iouh
    fx7 = x_top @ W_fx.T + b_fx + b_fh
    c_arr = np.zeros((15, M), np.float32)
    h_arr = np.zeros((15, M), np.float32)
    c_arr[7:15] = C_loc[:, 1]
    h_arr[7:15] = H_loc[:, 1]
    for j in range(6, -1, -1):
        ch = [2 * j + 1, 2 * j + 2]
        hs2 = h_arr[ch]
        iou = iou_x7[j] + (hs2[0] + hs2[1]) @ W_iouh.T
        i_g, o_g, u_g = np.split(iou, 3)
        i_g, o_g, u_g = sigmoid(i_g), sigmoid(o_g), np.tanh(u_g)
        f = sigmoid(hs2 @ W_fh.T + fx7[j])
        fcs = (f * c_arr[ch]).sum(0)
        c_arr[j] = i_g * u_g + fcs
        h_arr[j] = o_g * np.tanh(c_arr[j])
    return np.stack([c_arr[0], h_arr[0]]).astype(np.float32)


_LAST_IN_MAPS = None


# revision 15
# speedup vs baseline: 1.9272x; 1.0444x over previous
"""ChildSumTreeLSTM on a complete binary tree (N=8191), 8-core Trainium2.

v3: heap-ordered tree = 7 top nodes + 8 independent 1023-node subtrees,
one per NeuronCore. Per core, feature-major [256 feats x cols] layout with
col = subtree-local heap index (level l at cols [2^l, 2^(l+1)), leaves at
[512,1024)).

- x-projections in fp8e4m3 DoubleRow matmuls (K=256 in one instruction),
  computed just-in-time into the same PSUM accumulation group as each
  level's bf16 h-matmuls (scan weights pre-scaled x4096 to match the fp8
  input scaling; activations descale by 1/4096 and add biases via ports).
- f-gate x-terms use a host-duplicated x tensor (x8d[c] = x8[c//2]).
- Input DMA split so the leaf half arrives first; leaf matmuls start
  immediately.
- Dummy matmuls keep the PE busy through activation windows so it stays
  at the fast p-state.
- Device computes leaves + levels 256/128; the top of each subtree
  (<=64, 1023 nodes total + global top 7) finishes on host, vectorized.
"""

import numpy as np

import concourse.bass as bass
import concourse.tile as tile
from concourse import mybir
from concourse.bass_utils import run_bass_kernel_spmd

F32 = mybir.dt.float32
BF16 = mybir.dt.bfloat16
FP8 = mybir.dt.float8e4
AFT = mybir.ActivationFunctionType
DR = mybir.MatmulPerfMode.DoubleRow

N_NODES = 8191
D = 256
M = 256
NCOL = 1024
SUB_LEVELS = 10
DESCALE = 1.0 / 4096.0  # x8 = 128*x, wc8 = 32*Wc, wsc = 4096*W
DEV_LEVELS = (256, 128)  # internal levels computed on device
BOUND = 128              # boundary level emitted to host


def _split_excess_waits(nc, max_waits=1):
    """walrus in this container allows only 1 sync-wait per instruction.

    Tile can attach several; hoist the extras onto injected same-engine NOPs
    immediately preceding the instruction (same blocking semantics)."""
    k = 0
    for f in nc.m.functions:
        for bb in f.blocks:
            out = []
            changed = False
            for ins in bb.instructions:
                si = ins.sync_info
                w = list(si.on_wait) if si and si.on_wait else []
                if len(w) > max_waits:
                    hoist, keep = w[:-max_waits], w[-max_waits:]
                    for sw in hoist:
                        nop = mybir.InstNoOp(name=f"whoist{k}", ins=[], outs=[])
                        k += 1
                        nop.engine = ins.engine
                        nop.sync_info = mybir.SyncInfo(on_wait=[sw], on_update=[])
                        out.append(nop)
                    si.on_wait = keep
                    changed = True
                out.append(ins)
            if changed:
                bb.instructions = out


def _build_module():
    nc = bass.Bass(num_devices=8)

    # head8: wc8 [0:1024] | x8 leaf half [1024:1536] (leaf cols 512..1023)
    head8 = nc.dram_tensor("head8", [128, 2, 1536], FP8, kind="ExternalInput")
    # rest8: x8 internal cols 0..511 [0:512] | x8d cols 128..1023 [512:1408]
    rest8 = nc.dram_tensor("rest8", [128, 2, 1408], FP8, kind="ExternalInput")
    wsc = nc.dram_tensor("wsc", [128, 2, NCOL], BF16, kind="ExternalInput")
    biasd = nc.dram_tensor("biasd", [128, 16], F32, kind="ExternalInput")
    out_c = nc.dram_tensor("out_c", [128, 2 * BOUND], BF16, kind="ExternalOutput")
    out_h = nc.dram_tensor("out_h", [128, 2 * BOUND], BF16, kind="ExternalOutput")

    with tile.TileContext(nc) as tc:
        with (
            tc.tile_pool(name="consts", bufs=1) as consts,
            tc.tile_pool(name="tmps", bufs=2) as tmps,
            tc.tile_pool(name="spsum", bufs=1, space="PSUM") as spsum,
        ):
            sb_h8 = consts.tile([128, 2, 1536], FP8, tag="h8")
            nc.sync.dma_start(out=sb_h8[:], in_=head8[:])
            sb_b = consts.tile([128, 16], F32, tag="bias")
            nc.sync.dma_start(out=sb_b[:], in_=biasd[:])
            sb_r8 = consts.tile([128, 2, 1408], FP8, tag="r8")
            nc.sync.dma_start(out=sb_r8[:], in_=rest8[:])
            sb_wsc = consts.tile([128, 2, NCOL], BF16, tag="wsc")
            nc.sync.dma_start(out=sb_wsc[:], in_=wsc[:])

            wc8 = sb_h8[:, :, 0:1024]
            x8leaf = sb_h8[:, :, 1024:1536]    # leaf cols 512..1023
            x8int = sb_r8[:, :, 0:512]         # cols 0..511

            def x8d_ap(lo, hi):  # duplicated-parent cols lo..hi (128<=lo)
                return sb_r8[:, :, 512 + lo - 128 : 512 + hi - 128]

            H = consts.tile([128, 2, NCOL], BF16, tag="H")
            C = consts.tile([128, 2, NCOL], BF16, tag="C")

            # leaf gate tiles (cols 512..1023)
            l_si = consts.tile([128, 2, 512], BF16, tag="l_si")
            l_so = consts.tile([128, 2, 512], BF16, tag="l_so")
            l_tu = consts.tile([128, 2, 512], BF16, tag="l_tu")
            l_fc = consts.tile([128, 2, 512], BF16, tag="l_fc")
            l_iu = consts.tile([128, 2, 512], BF16, tag="l_iu")
            l_tc = consts.tile([128, 2, 512], BF16, tag="l_tc")
            hs = consts.tile([128, 2, 256], BF16, tag="hs_l")

            def ps_iou_tile(tag, name):
                return spsum.tile([128, 2, 256], F32, tag=tag, bufs=2, name=name)

            def ps_f_tile(name):
                return spsum.tile([128, 512], F32, tag="pf", bufs=2, name=name)

            def dummy_mms(count, target_ap):
                # PE p-state keepalive: throwaway fp8 MMs into a PSUM region
                # that the next real group resets with start=True.
                for _ in range(count):
                    nc.tensor.matmul(
                        target_ap, wc8[:, :, 0:128], wc8[:, :, 0:512],
                        start=True, stop=True, perf_mode=DR,
                        skip_group_check=True,
                    )

            # ---- leaf phase ----
            # F: 0,1=i  2,3=o  4,5=u  6,7=fx   (sub = F%2 feature half)
            leaf_ps = {}
            order = (0, 4, 6, 2, 1, 5, 7, 3)  # i0,u0,f0,o0, i1,u1,f1,o1
            for F in order:
                if F < 6:
                    ps = ps_iou_tile(["pi", "pi", "po", "po", "pu", "pu"][F], f"lps{F}")
                    ps_ap = ps[:, :, :]
                else:
                    ps = ps_f_tile(f"lps{F}")
                    ps_ap = ps[:, :]
                nc.tensor.matmul(
                    ps_ap, wc8[:, :, 128 * F : 128 * (F + 1)], x8leaf[:],
                    start=True, stop=True, perf_mode=DR,
                )
                leaf_ps[F] = ps_ap
            gate_of = {0: l_si, 1: l_si, 2: l_so, 3: l_so, 4: l_tu, 5: l_tu,
                       6: l_fc, 7: l_fc}
            func_of = {0: AFT.Sigmoid, 1: AFT.Sigmoid, 2: AFT.Sigmoid,
                       3: AFT.Sigmoid, 4: AFT.Tanh, 5: AFT.Tanh,
                       6: AFT.Sigmoid, 7: AFT.Sigmoid}
            bcol_of = {0: 6, 1: 7, 2: 8, 3: 9, 4: 10, 5: 11, 6: 14, 7: 15}

            def leaf_act(F):
                nc.scalar.activation(
                    gate_of[F][:, F % 2, :], leaf_ps[F], func_of[F],
                    bias=sb_b[:, bcol_of[F] : bcol_of[F] + 1], scale=DESCALE,
                )

            # j=0 chain
            for F in (0, 4, 6):
                leaf_act(F)
            nc.vector.tensor_mul(l_iu[:, 0], l_si[:, 0], l_tu[:, 0])
            nc.vector.tensor_add(C[:, 0, 512:1024], l_iu[:, 0], l_fc[:, 0])
            leaf_act(2)
            nc.scalar.activation(l_tc[:, 0, :], C[:, 0, 512:1024], AFT.Tanh)
            nc.vector.tensor_mul(H[:, 0, 512:1024], l_so[:, 0], l_tc[:, 0])
            nc.vector.tensor_add(
                hs[:, 0, :], H[:, 0, 512:1024:2], H[:, 0, 513:1024:2]
            )
            # j=1 chain (adds on gpsimd to keep DVE free)
            for F in (1, 5, 7):
                leaf_act(F)
            nc.vector.tensor_mul(l_iu[:, 1], l_si[:, 1], l_tu[:, 1])
            nc.vector.tensor_add(C[:, 1, 512:1024], l_iu[:, 1], l_fc[:, 1])
            leaf_act(3)
            nc.scalar.activation(l_tc[:, 1, :], C[:, 1, 512:1024], AFT.Tanh)
            nc.vector.tensor_mul(H[:, 1, 512:1024], l_so[:, 1], l_tc[:, 1])
            nc.vector.tensor_add(
                hs[:, 1, :], H[:, 1, 512:1024:2], H[:, 1, 513:1024:2]
            )

            # ---- internal levels ----
            first = True
            for n in DEV_LEVELS:
                a, b2 = n, 2 * n          # parent cols
                ca, cb = 2 * n, 4 * n     # child cols

                if not first:
                    nc.vector.tensor_add(
                        hs[:, 0, :n], H[:, 0, ca:cb:2], H[:, 0, ca + 1 : cb : 2]
                    )
                    nc.vector.tensor_add(
                        hs[:, 1, :n], H[:, 1, ca:cb:2], H[:, 1, ca + 1 : cb : 2]
                    )

                # JIT x-projections (no H dependency: run during prior acts)
                ps_f = []
                for h in range(2):
                    ps = ps_f_tile(f"psf{h}_{n}")
                    Fb = 768 + 128 * h
                    if h == 0:
                        dummy_mms(10, ps[:, 0:512])
                    nc.tensor.matmul(
                        ps[:, : 2 * n], wc8[:, :, Fb : Fb + 128], x8d_ap(ca, cb),
                        start=True, stop=False, perf_mode=DR,
                    )
                    ps_f.append(ps)
                ps_iou = []
                for pair in range(3):
                    ps = ps_iou_tile(["pi", "po", "pu"][pair], f"ps{'iou'[pair]}_{n}")
                    for sub in range(2):
                        F = 2 * pair + sub
                        nc.tensor.matmul(
                            ps[:, sub, :n],
                            wc8[:, :, 128 * F : 128 * (F + 1)],
                            x8int[:, :, a:b2],
                            start=True, stop=False, perf_mode=DR,
                        )
                    ps_iou.append(ps)

                # h-matmuls: all j=0 first, then j=1 (j=0 leaf chain is ready
                # earlier); within j: f, u, i, o
                for j in range(2):
                    last = j == 1
                    for h in range(2):
                        Fb = 768 + 128 * h
                        nc.tensor.matmul(
                            ps_f[h][:, : 2 * n],
                            sb_wsc[:, j, Fb : Fb + 128],
                            H[:, j, ca:cb],
                            start=False, stop=last,
                        )
                    for pair in (2, 0, 1):  # u, i, o
                        for sub in range(2):
                            F = 2 * pair + sub
                            nc.tensor.matmul(
                                ps_iou[pair][:, sub, :n],
                                sb_wsc[:, j, 128 * F : 128 * (F + 1)],
                                hs[:, j, :n],
                                start=False, stop=last,
                            )

                # acts + cell, sub-split so the c0/tanh_c0/h0 chain runs
                # under the sub-1 and o activations
                t_f = tmps.tile([128, 2, 512], BF16, tag="t_f")
                t_si = tmps.tile([128, 2, 256], BF16, tag="t_si")
                t_so = tmps.tile([128, 2, 256], BF16, tag="t_so")
                t_tu = tmps.tile([128, 2, 256], BF16, tag="t_tu")
                g = tmps.tile([128, 2, 512], BF16, tag="g")
                fc = tmps.tile([128, 2, 256], BF16, tag="fc")
                iu = tmps.tile([128, 2, 256], BF16, tag="iu")
                t_tc = tmps.tile([128, 2, 256], BF16, tag="t_tc")

                def act_f(h):
                    nc.scalar.activation(
                        t_f[:, h, : 2 * n], ps_f[h][:, : 2 * n], AFT.Sigmoid,
                        bias=sb_b[:, 12 + h : 13 + h], scale=DESCALE,
                    )

                def act_iou(pair, gate, func, sub):
                    F = 2 * pair + sub
                    nc.scalar.activation(
                        gate[:, sub, :n], ps_iou[pair][:, sub, :n], func,
                        bias=sb_b[:, F : F + 1], scale=DESCALE,
                    )

                def gfc(s):
                    nc.vector.tensor_mul(
                        g[:, s, : 2 * n], t_f[:, s, : 2 * n], C[:, s, ca:cb]
                    )
                    nc.vector.tensor_add(
                        fc[:, s, :n], g[:, s, 0 : 2 * n : 2], g[:, s, 1 : 2 * n : 2]
                    )

                def iuc(s):
                    nc.vector.tensor_mul(iu[:, s, :n], t_si[:, s, :n], t_tu[:, s, :n])
                    nc.vector.tensor_add(C[:, s, a:b2], iu[:, s, :n], fc[:, s, :n])

                act_f(0)
                act_f(1)
                gfc(0)
                act_iou(2, t_tu, AFT.Tanh, 0)
                act_iou(0, t_si, AFT.Sigmoid, 0)
                gfc(1)
                iuc(0)
                act_iou(2, t_tu, AFT.Tanh, 1)
                act_iou(0, t_si, AFT.Sigmoid, 1)
                iuc(1)
                act_iou(1, t_so, AFT.Sigmoid, 0)
                nc.scalar.activation(t_tc[:, 0, :n], C[:, 0, a:b2], AFT.Tanh)
                nc.vector.tensor_mul(H[:, 0, a:b2], t_so[:, 0, :n], t_tc[:, 0, :n])
                if n == BOUND:
                    nc.sync.dma_start(out=out_c[:, :], in_=C[:, :, BOUND : 2 * BOUND])
                act_iou(1, t_so, AFT.Sigmoid, 1)
                nc.scalar.activation(t_tc[:, 1, :n], C[:, 1, a:b2], AFT.Tanh)
                nc.vector.tensor_mul(H[:, 1, a:b2], t_so[:, 1, :n], t_tc[:, 1, :n])
                if n == BOUND:
                    nc.sync.dma_start(
                        out=out_h[:, 0:BOUND], in_=H[:, 0, BOUND : 2 * BOUND]
                    )
                first = False

            # ---- emit boundary ----
            nc.gpsimd.dma_start(
                out=out_h[:, BOUND : 2 * BOUND], in_=H[:, 1, BOUND : 2 * BOUND]
            )

    _split_excess_waits(nc)
    return nc


_NC_CACHE = None


def _get_module():
    global _NC_CACHE
    if _NC_CACHE is None:
        _NC_CACHE = _build_module()
    return _NC_CACHE


def _expected_children():
    j = (N_NODES - 1) - np.arange(N_NODES)
    internal = (2 * j + 1) < N_NODES
    ch0 = (N_NODES - 1) - (2 * j + 1)
    ch1 = (N_NODES - 1) - (2 * j + 2)
    children = np.stack(
        [np.where(internal, ch0, 0), np.where(internal, ch1, 0)], axis=1
    ).astype(np.int32)
    mask = np.stack([internal, internal], axis=1)
    return children, mask


def _reference_numpy(emb, W_ioux, b_ioux, W_iouh, b_iouh, W_fx, b_fx, W_fh, b_fh,
                     ops, children, child_mask):
    # generic fallback (matches reference.py) for unexpected tree structure
    def sigmoid(v):
        return 1.0 / (1.0 + np.exp(-v))

    N = ops.shape[0]
    Md = W_fh.shape[0]
    x = emb[ops]
    iou_x = x @ W_ioux.T + b_ioux
    fx_all = x @ W_fx.T + b_fx
    ones = np.ones((Md,), np.float32)
    leaf_fh = ones @ W_fh.T + b_fh
    maskf = child_mask.astype(np.float32)
    c_arr = np.zeros((N, Md), np.float32)
    h_arr = np.zeros((N, Md), np.float32)
    for t in range(N):
        idx = children[t]
        m = maskf[t][:, None]
        ch_c = c_arr[idx] * m
        ch_h = h_arr[idx] * m
        is_leaf = maskf[t].sum() == 0
        h_sum = ones if is_leaf else ch_h.sum(0)
        iou = iou_x[t] + h_sum @ W_iouh.T + b_iouh
        i, o, u = np.split(iou, 3)
        i, o, u = sigmoid(i), sigmoid(o), np.tanh(u)
        f = sigmoid(ch_h @ W_fh.T + b_fh + fx_all[t])
        fc_int = (f * ch_c).sum(0)
        fc_leaf = sigmoid(leaf_fh + fx_all[t])
        fc = fc_leaf if is_leaf else fc_int
        c = i * u + fc
        h = o * np.tanh(c)
        c_arr[t] = c
        h_arr[t] = h
    return np.stack([c_arr[N - 1], h_arr[N - 1]])


def _col_index_for_core(k):
    # col 0 pad; cols 1..1023: subtree-local heap order shifted by +1
    # (level l at cols [2^l, 2^(l+1)), leaves exactly at [512, 1024))
    idx = np.zeros(NCOL, np.int64)
    for l in range(SUB_LEVELS):
        n = 1 << l
        g0 = (1 << (3 + l)) - 1 + k * n
        idx[n : 2 * n] = g0 + np.arange(n)
    return idx


def _pack_fm(mat, dtype):
    # mat [cols, 256] -> [128, 2, cols]: out[p, j, c] = mat[c, j*128+p]
    cols = mat.shape[0]
    return np.ascontiguousarray(
        mat.T.reshape(2, 128, cols).transpose(1, 0, 2)
    ).astype(dtype)


def kernel(**inputs):
    emb = np.asarray(inputs["emb"], np.float32)
    W_ioux = np.asarray(inputs["W_ioux"], np.float32)
    b_ioux = np.asarray(inputs["b_ioux"], np.float32)
    W_iouh = np.asarray(inputs["W_iouh"], np.float32)
    b_iouh = np.asarray(inputs["b_iouh"], np.float32)
    W_fx = np.asarray(inputs["W_fx"], np.float32)
    b_fx = np.asarray(inputs["b_fx"], np.float32)
    W_fh = np.asarray(inputs["W_fh"], np.float32)
    b_fh = np.asarray(inputs["b_fh"], np.float32)
    ops = np.asarray(inputs["ops"], np.int32)
    children = np.asarray(inputs["children"], np.int32)
    child_mask = np.asarray(inputs["child_mask"])

    exp_children, exp_mask = _expected_children()
    if (
        ops.shape[0] != N_NODES
        or not np.array_equal(children, exp_children)
        or not np.array_equal(child_mask.astype(bool), exp_mask)
    ):
        return _reference_numpy(
            emb, W_ioux, b_ioux, W_iouh, b_iouh, W_fx, b_fx, W_fh, b_fh,
            ops, children, child_mask,
        )

    import ml_dtypes

    fp8 = ml_dtypes.float8_e4m3
    bf16 = ml_dtypes.bfloat16

    # ---- host prep ----
    x = emb[ops]          # [8191, 256] topo order
    x_heap = x[::-1]      # heap order (topo t = N-1-j)

    Wc = np.concatenate([W_ioux, W_fx], 0)       # [1024, 256]
    Ws = np.concatenate([W_iouh, W_fh], 0)       # [1024, 256]
    wc8 = _pack_fm(32.0 * Wc, fp8)               # [128, 2, 1024]
    wsc = _pack_fm(4096.0 * Ws, bf16)

    bias = np.zeros((128, 16), np.float32)
    bias[:, 0:6] = (b_ioux + b_iouh).reshape(6, 128).T
    bias[:, 6:12] = (b_ioux + W_iouh.sum(1) + b_iouh).reshape(6, 128).T
    bias[:, 12:14] = (b_fx + b_fh).reshape(2, 128).T
    bias[:, 14:16] = (b_fx + W_fh.sum(1) + b_fh).reshape(2, 128).T

    common = {"wsc": wsc, "biasd": bias}
    in_maps = []
    for k in range(8):
        idx = _col_index_for_core(k)
        xv = x_heap[idx]                          # [1024, 256]
        x8 = _pack_fm(128.0 * xv, fp8)
        head8 = np.ascontiguousarray(
            np.concatenate([wc8, x8[:, :, 512:1024]], axis=2)
        )
        x8d = x8[:, :, np.arange(128, 1024) // 2]
        rest8 = np.ascontiguousarray(
            np.concatenate([x8[:, :, 0:512], x8d], axis=2)
        )
        in_maps.append({"head8": head8, "rest8": rest8, **common})

    global _LAST_IN_MAPS
    _LAST_IN_MAPS = in_maps
    nc = _get_module()
    res = run_bass_kernel_spmd(nc, in_maps, list(range(8)))

    # ---- host: subtree levels 64..1 + global top 7 ----
    def sigmoid(v):
        return 1.0 / (1.0 + np.exp(-v))

    # unpack boundary: [128, 2*BOUND] -> [BOUND nodes, 256 feats]
    C_loc = np.zeros((8, 2 * BOUND, M), np.float32)
    H_loc = np.zeros((8, 2 * BOUND, M), np.float32)
    for k in range(8):
        rc = res.results[k]["out_c"]
        rh = res.results[k]["out_h"].astype(np.float32)
        C_loc[k, BOUND:] = rc.reshape(128, 2, BOUND).transpose(2, 1, 0).reshape(BOUND, M)
        H_loc[k, BOUND:] = rh.reshape(128, 2, BOUND).transpose(2, 1, 0).reshape(BOUND, M)

    # x-projections for host nodes (cols 1..BOUND-1 per core + global top 7)
    nb = BOUND - 1
    idx_all = np.stack([_col_index_for_core(k)[1:BOUND] for k in range(8)])
    x_host = x_heap[idx_all.reshape(-1)].astype(np.float32)
    iou_xh = (x_host @ W_ioux.T + b_ioux + b_iouh).reshape(8, nb, 3 * M)
    fx_h = (x_host @ W_fx.T + b_fx + b_fh).reshape(8, nb, M)

    n = BOUND // 2
    while n >= 1:
        ch_h = H_loc[:, 2 * n : 4 * n]            # [8, 2n, 256]
        ch_c = C_loc[:, 2 * n : 4 * n]
        hsum = ch_h[:, 0::2] + ch_h[:, 1::2]      # [8, n, 256]
        iou = iou_xh[:, n - 1 : 2 * n - 1] + hsum @ W_iouh.T
        i_g = sigmoid(iou[:, :, :M])
        o_g = sigmoid(iou[:, :, M : 2 * M])
        u_g = np.tanh(iou[:, :, 2 * M :])
        fxd = np.repeat(fx_h[:, n - 1 : 2 * n - 1], 2, axis=1)
        f = sigmoid(ch_h @ W_fh.T + fxd)
        gfc = f * ch_c
        fcs = gfc[:, 0::2] + gfc[:, 1::2]
        c = i_g * u_g + fcs
        C_loc[:, n : 2 * n] = c
        H_loc[:, n : 2 * n] = o_g * np.tanh(c)
        n //= 2

    # global top 15: nodes 7..14 are the subtree roots (core k -> 7+k)
    x_top = x_heap[0:7].astype(np.float32)
    iou_x7 = x_top @ W_ioux.T + b_ioux + b_iouh
    fx7 = x_top @ W_fx.T + b_fx + b_fh
    c_arr = np.zeros((15, M), np.float32)
    h_arr = np.zeros((15, M), np.float32)
    c_arr[7:15] = C_loc[:, 1]
    h_arr[7:15] = H_loc[:, 1]
    for j in range(6, -1, -1):
        ch = [2 * j + 1, 2 * j + 2]
        hs2 = h_arr[ch]
        iou = iou_x7[j] + (hs2[0] + hs2[1]) @ W_iouh.T
        i_g, o_g, u_g = np.split(iou, 3)
        i_g, o_g, u_g = sigmoid(i_g), sigmoid(o_g), np.tanh(u_g)
        f = sigmoid(hs2 @ W_fh.T + fx7[j])
        fcs = (f * c_arr[ch]).sum(0)
        c_arr[j] = i_g * u_g + fcs
        h_arr[j] = o_g * np.tanh(c_arr[j])
    return np.stack([c_arr[0], h_arr[0]]).astype(np.float32)


_LAST_IN_MAPS = None
